# revision 14
# baseline (speedup 1.0000x reference)
"""MinGRU block kernel for Trainium2 (Bass/Tile), 8-core data-parallel over batch.

Reference computation (per batch b):
    xn = rmsnorm(x, w_rms_mix)
    g = xn@Wg+bg; v = xn@Wv+bv; d = xn@Wd+bd
    x_scan = sigmoid(g)*tanh(v);  a = 0.001 + 0.998*sigmoid(d)
    h = linear_scan(x_scan, a)          # h_t = a_t h_{t-1} + x_t along S
    x2 = x + h
    yn = rmsnorm(x2, w_rms_ffn)
    out = x2 + (silu(yn@W_gate) * (yn@W_up)) @ W_out

Shapes: B=8, S=4096, D=1024, F=3072 (fp32).  Each core handles one batch.

Design notes:
  - All matmul activations live in transposed layout [feature, token] so the
    contraction dim (features) is on partitions; weights are used directly as
    lhsT in their natural [in, out] storage.  The sequential scan runs along
    the free (token) axis via the DVE tensor_tensor_scan instruction.
  - Phase 1 (mixer): transpose x (PE), g/v/d matmuls, sigmoid/tanh epilogue,
    scan with carried state across token blocks, x2^T = x^T + h^T, plus the
    FFN-norm sum-of-squares (ones-matmul over partitions).  x2^T spills to
    DRAM.
  - Phase 2a: gate/up matmuls from x2n^T, silu*up -> hidden^T (spills).
  - Phase 2b: out matmul, residual add, PE transpose-back to natural layout.
  - rsqrt for rmsnorm: ACT Sqrt + DVE reciprocal + one Newton step (ACT Rsqrt
    is banned for accuracy).  Sqrt calls are hoisted/batched so the ACT table
    set switches only ~4 times total.
  - Weights are pre-folded with the rms weight vectors and cast to bf16 on
    host; matmuls run in bf16 (fp32 accumulation in PSUM).
"""

import sys

for _p in ("/opt/trn_rl_repo", "/root/.axon_site/_ro/trn_rl_repo"):
    if _p not in sys.path:
        sys.path.insert(0, _p)

from contextlib import ExitStack
from dataclasses import dataclass

import ml_dtypes
import numpy as np

import concourse.bass as bass
import concourse.tile as tile
from concourse import bacc, mybir
from concourse.masks import make_identity

F32 = mybir.dt.float32
BF16 = mybir.dt.bfloat16
AF = mybir.ActivationFunctionType
ALU = mybir.AluOpType

EPS = 1e-6


@dataclass(frozen=True)
class Cfg:
    S: int = 4096
    D: int = 1024
    F: int = 3072
    Tb: int = 256  # token block (matmul moving free dim)

    @property
    def NB(self):
        return self.S // self.Tb

    @property
    def TC(self):
        return self.Tb // 128  # token chunks per block

    @property
    def KD(self):
        return self.D // 128  # D in 128-chunks

    @property
    def KF(self):
        return self.F // 128  # F in 128-chunks


def _rsqrt(nc, pool, ms, shape, tag):
    """rms = 1/sqrt(ms) with one Newton refinement. ms is an f32 AP.

    Returns an f32 tile of `shape`.
    """
    s = pool.tile(shape, F32, tag=f"{tag}_s", name=f"{tag}_s")
    nc.scalar.activation(s, ms, AF.Sqrt)
    r0 = pool.tile(shape, F32, tag=f"{tag}_r0", name=f"{tag}_r0")
    nc.vector.reciprocal(r0, s)
    # newton: r = r0 * (1.5 - 0.5 * ms * r0^2)
    t1 = pool.tile(shape, F32, tag=f"{tag}_t1", name=f"{tag}_t1")
    nc.vector.tensor_mul(t1, r0, r0)
    nc.vector.tensor_mul(t1, t1, ms)
    nc.vector.tensor_scalar(t1, t1, -0.5, 1.5, op0=ALU.mult, op1=ALU.add)
    r = pool.tile(shape, F32, tag=f"{tag}_r", name=f"{tag}_r")
    nc.vector.tensor_mul(r, r0, t1)
    return r


def build_mingru(tc: tile.TileContext, outs: dict, ins: dict, cfg: Cfg):
    nc = tc.nc
    S, D, F_, Tb = cfg.S, cfg.D, cfg.F, cfg.Tb
    NB, TC, KD, KF = cfg.NB, cfg.TC, cfg.KD, cfg.KF

    x = ins["x"]  # [S, D] f32
    wg, wv, wd = ins["wg"], ins["wv"], ins["wd"]  # [D, D] bf16 (rms-folded)
    bg, bv, bd = ins["bg"], ins["bv"], ins["bd"]  # [KD, 128] f32
    wgate, wup = ins["wgate"], ins["wup"]  # [D, F] bf16 (rms-folded)
    wout = ins["wout"]  # [F, D] bf16
    out = outs["out"]  # [S, D] f32

    ctx = ExitStack()
    with ctx:
        singles = ctx.enter_context(tc.tile_pool(name="singles", bufs=1))
        dram = ctx.enter_context(tc.tile_pool(name="dram", bufs=1, space="DRAM"))

        ident = singles.tile([128, 128], F32)
        make_identity(nc, ident)
        ones_row = singles.tile([1, 128], F32)
        nc.gpsimd.memset(ones_row, 1.0)
        ones_col = singles.tile([128, 1], BF16)
        nc.gpsimd.memset(ones_col, 1.0)

        # biases as [128, KD] so bias[:, m] is a per-partition scalar AP
        bgs = singles.tile([128, KD], F32)
        bvs = singles.tile([128, KD], F32)
        bds = singles.tile([128, KD], F32)
        nc.sync.dma_start(out=bgs, in_=bg.rearrange("m p -> p m"))
        nc.sync.dma_start(out=bvs, in_=bv.rearrange("m p -> p m"))
        nc.sync.dma_start(out=bds, in_=bd.rearrange("m p -> p m"))

        # DRAM scratch
        x2t_d = dram.tile([D, S], F32)
        ss2_d = dram.tile([1, S], F32)
        hid_d = dram.tile([F_, S], BF16)

        nchunks = S // 128

        # pool spanning prepass + phase 1 (released before FFN weights load)
        mixspan = tc.alloc_tile_pool(name="mixspan", bufs=1)

        # ---------------- prepass: rms1 for all tokens ----------------
        rms1_row = mixspan.tile([1, S], F32)  # token-indexed row on partition 0
        with tc.tile_pool(name="pre", bufs=3) as pre, tc.tile_pool(
            name="pre_ps", bufs=2, space="PSUM"
        ) as pre_ps:
            ss1 = singles.tile([128, nchunks], F32)
            for i in range(nchunks):
                xpre = pre.tile([128, D], F32, tag="xpre")
                nc.sync.dma_start(out=xpre, in_=x[i * 128 : (i + 1) * 128, :])
                sqdump = pre.tile([128, D], BF16, tag="sqdump")
                nc.scalar.activation(
                    sqdump, xpre, AF.Square, accum_out=ss1[:, i : i + 1]
                )
            ms1 = singles.tile([128, nchunks], F32)
            nc.vector.tensor_scalar(
                ms1, ss1, 1.0 / D, EPS, op0=ALU.mult, op1=ALU.add
            )
            rms1 = _rsqrt(nc, pre, ms1, [128, nchunks], "rms1")
            # transpose each [128,1] column to a [1,128] row at partition 0
            for i in range(nchunks):
                ptrow = pre_ps.tile([1, 128], F32, tag="ptrow", name="ptrow")
                nc.tensor.transpose(ptrow, rms1[:, i : i + 1], ident)
                nc.vector.tensor_copy(rms1_row[0:1, i * 128 : (i + 1) * 128], ptrow)

        # ---------------- phase 1: mixer ----------------
        prev_h = {}
        with tc.tile_pool(name="wmix", bufs=1) as wmix, tc.tile_pool(
            name="p1", bufs=2
        ) as p1, tc.tile_pool(name="p1h", bufs=2) as p1h, tc.tile_pool(
            name="ps_tr", bufs=2, space="PSUM"
        ) as ps_tr, tc.tile_pool(
            name="ps_gvd", bufs=1, space="PSUM"
        ) as ps_gvd, tc.tile_pool(
            name="ps_ss2", bufs=2, space="PSUM"
        ) as ps_ss2:
            # mixer weights resident: [128, D] bf16 per k-chunk
            wg_sb = [wmix.tile([128, D], BF16, tag=f"wg{k}", name=f"wg{k}") for k in range(KD)]
            wv_sb = [wmix.tile([128, D], BF16, tag=f"wv{k}", name=f"wv{k}") for k in range(KD)]
            wd_sb = [wmix.tile([128, D], BF16, tag=f"wd{k}", name=f"wd{k}") for k in range(KD)]
            for k in range(KD):
                nc.sync.dma_start(out=wg_sb[k], in_=wg[k * 128 : (k + 1) * 128, :])
                nc.sync.dma_start(out=wv_sb[k], in_=wv[k * 128 : (k + 1) * 128, :])
                nc.sync.dma_start(out=wd_sb[k], in_=wd[k * 128 : (k + 1) * 128, :])

            for j in range(NB):
                t0 = j * Tb
                # load x block as [p, c, d]
                xblk = p1.tile([128, TC, D], F32, tag="xblk", name="xblk")
                nc.sync.dma_start(
                    out=xblk,
                    in_=x[t0 : t0 + Tb, :].rearrange("(c p) d -> p c d", p=128),
                )

                # rms1 broadcast for this block: [128, Tb] psum
                rb = ps_tr.tile([128, Tb], F32, tag="rb", bufs=1, name="rb")
                for c in range(TC):
                    tt = t0 + c * 128
                    nc.tensor.matmul(
                        rb[:, c * 128 : (c + 1) * 128],
                        lhsT=ones_row,
                        rhs=rms1_row[0:1, tt : tt + 128],
                        start=True,
                        stop=True,
                    )

                # transpose x -> x^T tiles, and xn^T = x^T * rms1 (bf16)
                xT = []
                xnT = []
                for m in range(KD):
                    pt = ps_tr.tile([128, Tb], F32, tag="ptr", name="ptr")
                    for c in range(TC):
                        nc.tensor.transpose(
                            pt[:, c * 128 : (c + 1) * 128],
                            xblk[:, c, m * 128 : (m + 1) * 128],
                            ident,
                        )
                    xT_m = p1.tile([128, Tb], F32, tag=f"xT{m}", name=f"xT{m}")
                    nc.vector.tensor_copy(xT_m, pt)
                    xnT_m = p1.tile([128, Tb], BF16, tag=f"xnT{m}", name=f"xnT{m}")
                    nc.vector.tensor_mul(xnT_m, xT_m, rb)
                    xT.append(xT_m)
                    xnT.append(xnT_m)

                # mixer matmuls + epilogue + scan, per output d-chunk
                for m in range(KD):
                    psg = ps_gvd.tile([128, Tb], F32, tag="psg", name="psg")
                    psv = ps_gvd.tile([128, Tb], F32, tag="psv", name="psv")
                    psd = ps_gvd.tile([128, Tb], F32, tag="psd", name="psd")
                    for k in range(KD):
                        st, sp = (k == 0), (k == KD - 1)
                        nc.tensor.matmul(
                            psg, lhsT=wg_sb[k][:, m * 128 : (m + 1) * 128],
                            rhs=xnT[k], start=st, stop=sp,
                        )
                        nc.tensor.matmul(
                            psv, lhsT=wv_sb[k][:, m * 128 : (m + 1) * 128],
                            rhs=xnT[k], start=st, stop=sp,
                        )
                        nc.tensor.matmul(
                            psd, lhsT=wd_sb[k][:, m * 128 : (m + 1) * 128],
                            rhs=xnT[k], start=st, stop=sp,
                        )
                    sg = p1.tile([128, Tb], F32, tag="sg", name="sg")
                    nc.scalar.activation(sg, psg, AF.Sigmoid, bias=bgs[:, m : m + 1])
                    tv = p1.tile([128, Tb], F32, tag="tv", name="tv")
                    nc.scalar.activation(tv, psv, AF.Tanh, bias=bvs[:, m : m + 1])
                    sd = p1.tile([128, Tb], F32, tag="sd", name="sd")
                    nc.scalar.activation(sd, psd, AF.Sigmoid, bias=bds[:, m : m + 1])

                    xs = p1.tile([128, Tb], F32, tag="xs", name="xs")
                    nc.vector.tensor_mul(xs, sg, tv)
                    aa = p1.tile([128, Tb], F32, tag="aa", name="aa")
                    nc.vector.tensor_scalar(
                        aa, sd, 0.998, 0.001, op0=ALU.mult, op1=ALU.add
                    )

                    h_m = p1h.tile([128, Tb], F32, tag=f"h{m}", name=f"h{m}")
                    init = 0.0 if j == 0 else prev_h[m][:, Tb - 1 : Tb]
                    nc.vector.tensor_tensor_scan(
                        h_m, data0=aa, data1=xs, initial=init,
                        op0=ALU.mult, op1=ALU.add,
                    )
                    prev_h[m] = h_m

                    x2T_m = p1.tile([128, Tb], F32, tag=f"x2T{m}", name=f"x2T{m}")
                    nc.vector.tensor_add(x2T_m, xT[m], h_m)
                    nc.sync.dma_start(
                        out=x2t_d[m * 128 : (m + 1) * 128, t0 : t0 + Tb],
                        in_=x2T_m,
                    )
                    # FFN-norm sum of squares: ones^T @ x2^2 accumulated over m
                    sq2 = p1.tile([128, Tb], BF16, tag="sq2", name="sq2")
                    nc.scalar.activation(sq2, x2T_m, AF.Square)
                    ss2_ps = (
                        ps_ss2.tile([1, Tb], F32, tag="ss2", name="ss2")
                        if m == 0
                        else ss2_ps
                    )
                    nc.tensor.matmul(
                        ss2_ps, lhsT=ones_col, rhs=sq2,
                        start=(m == 0), stop=(m == KD - 1),
                    )
                ss2_sb = p1.tile([1, Tb], F32, tag="ss2sb", name="ss2sb")
                nc.vector.tensor_copy(ss2_sb, ss2_ps)
                nc.sync.dma_start(out=ss2_d[:, t0 : t0 + Tb], in_=ss2_sb)

        mixspan.release()

        # ---------------- phase 2a: gate/up -> hidden ----------------
        with tc.tile_pool(name="wglu", bufs=1) as wglu, tc.tile_pool(
            name="p2a", bufs=2
        ) as p2a, tc.tile_pool(name="ps_2a", bufs=2, space="PSUM") as ps_2a:
            wgate_sb = [wglu.tile([128, F_], BF16, tag=f"wgate{k}", name=f"wgate{k}") for k in range(KD)]
            wup_sb = [wglu.tile([128, F_], BF16, tag=f"wup{k}", name=f"wup{k}") for k in range(KD)]
            for k in range(KD):
                nc.sync.dma_start(out=wgate_sb[k], in_=wgate[k * 128 : (k + 1) * 128, :])
                nc.sync.dma_start(out=wup_sb[k], in_=wup[k * 128 : (k + 1) * 128, :])

            # rms2 for all tokens, chunked (keeps the batched-sqrt table
            # behavior but avoids several [1, S] temps, which each reserve
            # free-size bytes on every partition)
            rms2_row = p2a.tile([1, S], F32, tag="rms2_row", bufs=1, name="rms2_row")
            RCH = min(S, 512)
            for q in range(S // RCH):
                rsl = slice(q * RCH, (q + 1) * RCH)
                ssc = p2a.tile([1, RCH], F32, tag="ssc", name="ssc")
                nc.sync.dma_start(out=ssc, in_=ss2_d[:, rsl])
                msc = p2a.tile([1, RCH], F32, tag="msc", name="msc")
                nc.vector.tensor_scalar(
                    msc, ssc, 1.0 / D, EPS, op0=ALU.mult, op1=ALU.add
                )
                rr = _rsqrt(nc, p2a, msc, [1, RCH], "rms2")
                nc.vector.tensor_copy(rms2_row[0:1, rsl], rr)

            for j in range(NB):
                t0 = j * Tb
                rb2 = ps_2a.tile([128, Tb], F32, tag="rb2", name="rb2")
                for c in range(TC):
                    tt = t0 + c * 128
                    nc.tensor.matmul(
                        rb2[:, c * 128 : (c + 1) * 128],
                        lhsT=ones_row,
                        rhs=rms2_row[0:1, tt : tt + 128],
                        start=True,
                        stop=True,
                    )
                x2nT = []
                for m in range(KD):
                    x2a_m = p2a.tile([128, Tb], F32, tag=f"x2a{m}", name=f"x2a{m}")
                    nc.sync.dma_start(
                        out=x2a_m,
                        in_=x2t_d[m * 128 : (m + 1) * 128, t0 : t0 + Tb],
                    )
                    x2nT_m = p2a.tile([128, Tb], BF16, tag=f"x2nT{m}", name=f"x2nT{m}")
                    nc.vector.tensor_mul(x2nT_m, x2a_m, rb2)
                    x2nT.append(x2nT_m)

                for f in range(KF):
                    pg = ps_2a.tile([128, Tb], F32, tag="pg", name="pg")
                    pu = ps_2a.tile([128, Tb], F32, tag="pu", name="pu")
                    for k in range(KD):
                        st, sp = (k == 0), (k == KD - 1)
                        nc.tensor.matmul(
                            pg, lhsT=wgate_sb[k][:, f * 128 : (f + 1) * 128],
                            rhs=x2nT[k], start=st, stop=sp,
                        )
                        nc.tensor.matmul(
                            pu, lhsT=wup_sb[k][:, f * 128 : (f + 1) * 128],
                            rhs=x2nT[k], start=st, stop=sp,
                        )
                    # silu(g) = g * sigmoid(g), composed so each DVE op reads
                    # at most one PSUM operand
                    sl = p2a.tile([128, Tb], F32, tag="sl", name="sl")
                    nc.scalar.activation(sl, pg, AF.Sigmoid)
                    sl2 = p2a.tile([128, Tb], F32, tag="sl2", name="sl2")
                    nc.vector.tensor_mul(sl2, sl, pg)
                    hid = p2a.tile([128, Tb], BF16, tag="hid", bufs=3, name="hid")
                    nc.vector.tensor_mul(hid, sl2, pu)
                    nc.sync.dma_start(
                        out=hid_d[f * 128 : (f + 1) * 128, t0 : t0 + Tb], in_=hid
                    )

        # ---------------- phase 2b: out matmul + residual + transpose ----------------
        with tc.tile_pool(name="wo", bufs=1) as wo, tc.tile_pool(
            name="p2b", bufs=2
        ) as p2b, tc.tile_pool(name="ps_2b", bufs=2, space="PSUM") as ps_2b:
            wout_sb = [wo.tile([128, D], BF16, tag=f"wout{k}", name=f"wout{k}") for k in range(KF)]
            for k in range(KF):
                nc.sync.dma_start(out=wout_sb[k], in_=wout[k * 128 : (k + 1) * 128, :])

            for j in range(NB):
                t0 = j * Tb
                hidT = []
                for k in range(KF):
                    hT_k = p2b.tile([128, Tb], BF16, tag=f"hT{k}", name=f"hT{k}")
                    nc.sync.dma_start(
                        out=hT_k, in_=hid_d[k * 128 : (k + 1) * 128, t0 : t0 + Tb]
                    )
                    hidT.append(hT_k)
                outT = []
                for m in range(KD):
                    x2b_m = p2b.tile([128, Tb], F32, tag=f"x2b{m}", name=f"x2b{m}")
                    nc.sync.dma_start(
                        out=x2b_m,
                        in_=x2t_d[m * 128 : (m + 1) * 128, t0 : t0 + Tb],
                    )
                    pf = ps_2b.tile([128, Tb], F32, tag="pf", name="pf")
                    for k in range(KF):
                        nc.tensor.matmul(
                            pf, lhsT=wout_sb[k][:, m * 128 : (m + 1) * 128],
                            rhs=hidT[k], start=(k == 0), stop=(k == KF - 1),
                        )
                    outT_m = p2b.tile([128, Tb], F32, tag=f"outT{m}", name=f"outT{m}")
                    nc.vector.tensor_add(outT_m, x2b_m, pf)
                    outT.append(outT_m)
                # transpose back to natural [token, D] and store
                for c in range(TC):
                    obl = p2b.tile([128, D], F32, tag="obl", bufs=3, name="obl")
                    for m in range(KD):
                        pt2 = ps_2b.tile([128, 128], F32, tag="pt2", name="pt2")
                        nc.tensor.transpose(
                            pt2, outT[m][:, c * 128 : (c + 1) * 128], ident
                        )
                        nc.vector.tensor_copy(obl[:, m * 128 : (m + 1) * 128], pt2)
                    tt = t0 + c * 128
                    nc.sync.dma_start(out=out[tt : tt + 128, :], in_=obl)


# ----------------------------------------------------------------------------
# host side
# ----------------------------------------------------------------------------

def prep_weights(inputs: dict, cfg: Cfg):
    """Fold rms weight vectors into the matmul weights, cast to bf16, and
    reshape biases. Returns the per-core common input dict (everything except
    x)."""
    bf = ml_dtypes.bfloat16
    w_mix = np.asarray(inputs["w_rms_mix"], np.float32)[:, None]
    w_ffn = np.asarray(inputs["w_rms_ffn"], np.float32)[:, None]
    KD = cfg.D // 128
    return {
        "wg": (w_mix * np.asarray(inputs["Wg"], np.float32)).astype(bf),
        "wv": (w_mix * np.asarray(inputs["Wv"], np.float32)).astype(bf),
        "wd": (w_mix * np.asarray(inputs["Wd"], np.float32)).astype(bf),
        "bg": np.ascontiguousarray(
            np.asarray(inputs["bg"], np.float32).reshape(KD, 128)
        ),
        "bv": np.ascontiguousarray(
            np.asarray(inputs["bv"], np.float32).reshape(KD, 128)
        ),
        "bd": np.ascontiguousarray(
            np.asarray(inputs["bd"], np.float32).reshape(KD, 128)
        ),
        "wgate": (w_ffn * np.asarray(inputs["W_gate"], np.float32)).astype(bf),
        "wup": (w_ffn * np.asarray(inputs["W_up"], np.float32)).astype(bf),
        "wout": np.asarray(inputs["W_out"], np.float32).astype(bf),
    }


def build_nc(cfg: Cfg):
    bf = mybir.dt.bfloat16
    # Bacc (not bare Bass): its compile() pass splits multi-wait sync into
    # event semaphores (HW allows at most 1 wait per instruction) and
    # hoists ACT table loads.
    nc = bacc.Bacc("TRN2", target_bir_lowering=False, debug=False)
    KD = cfg.D // 128
    ins = {
        "x": nc.declare_dram_parameter("x", [cfg.S, cfg.D], F32, isOutput=False),
        "wg": nc.declare_dram_parameter("wg", [cfg.D, cfg.D], bf, isOutput=False),
        "wv": nc.declare_dram_parameter("wv", [cfg.D, cfg.D], bf, isOutput=False),
        "wd": nc.declare_dram_parameter("wd", [cfg.D, cfg.D], bf, isOutput=False),
        "bg": nc.declare_dram_parameter("bg", [KD, 128], F32, isOutput=False),
        "bv": nc.declare_dram_parameter("bv", [KD, 128], F32, isOutput=False),
        "bd": nc.declare_dram_parameter("bd", [KD, 128], F32, isOutput=False),
        "wgate": nc.declare_dram_parameter("wgate", [cfg.D, cfg.F], bf, isOutput=False),
        "wup": nc.declare_dram_parameter("wup", [cfg.D, cfg.F], bf, isOutput=False),
        "wout": nc.declare_dram_parameter("wout", [cfg.F, cfg.D], bf, isOutput=False),
    }
    outs = {
        "out": nc.declare_dram_parameter("out", [cfg.S, cfg.D], F32, isOutput=True),
    }
    ins_ap = {k: v.ap() for k, v in ins.items()}
    outs_ap = {k: v.ap() for k, v in outs.items()}
    with tile.TileContext(nc) as tc:
        build_mingru(tc, outs_ap, ins_ap, cfg)
    nc.compile()
    return nc


_NC_CACHE = {}


def kernel(**inputs) -> np.ndarray:
    from concourse.bass_utils import run_bass_kernel_spmd

    cfg = Cfg()
    x = np.asarray(inputs["x"], np.float32)  # [B, S, D]
    B = x.shape[0]
    common = prep_weights(inputs, cfg)

    if cfg not in _NC_CACHE:
        _NC_CACHE[cfg] = build_nc(cfg)
    nc = _NC_CACHE[cfg]

    in_maps = [dict(common, x=np.ascontiguousarray(x[b])) for b in range(B)]
    res = run_bass_kernel_spmd(nc, in_maps, core_ids=list(range(B)))
    out = np.stack([np.asarray(res.results[b]["out"]) for b in range(B)], axis=0)
    return out.astype(np.float32)


def _ensure_ntff_hook():
    """Register the axon NTFF profile hook if the agent image's antenv lacks
    axon_hooks (same ctypes shim trn_boot would install)."""
    import contextlib
    import ctypes
    import types

    try:
        from antenv.axon_hooks import get_axon_ntff_profile_hook

        if get_axon_ntff_profile_hook() is not None:
            return
    except ImportError:
        pass

    so_path = "/opt/axon/libaxon_pjrt.so"
    lib = ctypes.CDLL(so_path)
    if not hasattr(lib, "axon_start_nrt_profile"):
        return
    lib.axon_start_nrt_profile.argtypes = [
        ctypes.POINTER(ctypes.c_int64),
        ctypes.c_size_t,
    ]
    lib.axon_start_nrt_profile.restype = ctypes.c_int64
    lib.axon_stop_nrt_profile.argtypes = [ctypes.c_char_p]
    lib.axon_stop_nrt_profile.restype = ctypes.c_int64

    @contextlib.contextmanager
    def _hook(output_dir, device_ids):
        import jax

        jax.devices()
        if device_ids:
            ids = (ctypes.c_int64 * len(device_ids))(*device_ids)
            rc = lib.axon_start_nrt_profile(ids, len(device_ids))
        else:
            rc = lib.axon_start_nrt_profile(None, 0)
        if rc != 0:
            raise RuntimeError(f"axon_start_nrt_profile rc={rc}")
        try:
            yield
        finally:
            n = lib.axon_stop_nrt_profile(str(output_dir).encode())
            print(f"profile: {n} file(s) written to {output_dir}")

    mod = types.ModuleType("antenv.axon_hooks")
    mod.get_axon_ntff_profile_hook = lambda: _hook
    mod.set_axon_ntff_profile_hook = lambda h: None
    sys.modules["antenv.axon_hooks"] = mod
    import antenv

    antenv.axon_hooks = mod


def kernel_profiled(**inputs):
    """Run once with NTFF tracing; returns exec_time_ns (max across cores)."""
    from concourse import bass_utils
    from concourse.bass_utils import run_bass_kernel_spmd

    _ensure_ntff_hook()
    # skip the bucket upload (no creds needed for local analysis)
    bass_utils.upload_artifacts = lambda tmpdir: f"local:{tmpdir}"

    cfg = Cfg()
    x = np.asarray(inputs["x"], np.float32)
    B = x.shape[0]
    common = prep_weights(inputs, cfg)
    if cfg not in _NC_CACHE:
        _NC_CACHE[cfg] = build_nc(cfg)
    nc = _NC_CACHE[cfg]
    in_maps = [dict(common, x=np.ascontiguousarray(x[b])) for b in range(B)]
    import os
    tmpdir = "/tmp/mingru_profile"
    os.makedirs(tmpdir, exist_ok=True)
    res = run_bass_kernel_spmd(
        nc, in_maps, core_ids=list(range(B)), trace=True, tmpdir=tmpdir
    )
    return res.exec_time_ns


if __name__ == "__main__":
    rng = np.random.default_rng(0)
    cfg = Cfg()
    fake = {
        "x": rng.standard_normal((8, cfg.S, cfg.D), dtype=np.float32),
        "w_rms_mix": np.ones(cfg.D, np.float32),
        "w_rms_ffn": np.ones(cfg.D, np.float32),
        "Wg": rng.standard_normal((cfg.D, cfg.D), dtype=np.float32) / 32,
        "bg": np.zeros(cfg.D, np.float32),
        "Wv": rng.standard_normal((cfg.D, cfg.D), dtype=np.float32) / 32,
        "bv": np.zeros(cfg.D, np.float32),
        "Wd": rng.standard_normal((cfg.D, cfg.D), dtype=np.float32) / 32,
        "bd": np.ones(cfg.D, np.float32),
        "W_gate": rng.standard_normal((cfg.D, cfg.F), dtype=np.float32) / 32,
        "W_up": rng.standard_normal((cfg.D, cfg.F), dtype=np.float32) / 32,
        "W_out": rng.standard_normal((cfg.F, cfg.D), dtype=np.float32) / 55,
    }
    out = kernel(**fake)
    print(out.shape, out.dtype)


# revision 22
# speedup vs baseline: 1.0515x; 1.0515x over previous
"""MinGRU block kernel for Trainium2 (Bass/Tile), 8-core data-parallel over batch.

Reference computation (per batch b):
    xn = rmsnorm(x, w_rms_mix)
    g = xn@Wg+bg; v = xn@Wv+bv; d = xn@Wd+bd
    x_scan = sigmoid(g)*tanh(v);  a = 0.001 + 0.998*sigmoid(d)
    h = linear_scan(x_scan, a)          # h_t = a_t h_{t-1} + x_t along S
    x2 = x + h
    yn = rmsnorm(x2, w_rms_ffn)
    out = x2 + (silu(yn@W_gate) * (yn@W_up)) @ W_out

Shapes: B=8, S=4096, D=1024, F=3072 (fp32).  Each core handles one batch.

Design notes:
  - All matmul activations live in transposed layout [feature, token] so the
    contraction dim (features) is on partitions; weights are used directly as
    lhsT in their natural [in, out] storage.  The sequential scan runs along
    the free (token) axis via the DVE tensor_tensor_scan instruction.
  - Phase 1 (mixer): transpose x (PE), g/v/d matmuls, sigmoid/tanh epilogue,
    scan with carried state across token blocks, x2^T = x^T + h^T, plus the
    FFN-norm sum-of-squares (ones-matmul over partitions).  x2^T spills to
    DRAM.
  - Phase 2a: gate/up matmuls from x2n^T, silu*up -> hidden^T (spills).
  - Phase 2b: out matmul, residual add, PE transpose-back to natural layout.
  - rsqrt for rmsnorm: ACT Sqrt + DVE reciprocal + one Newton step (ACT Rsqrt
    is banned for accuracy).  Sqrt calls are hoisted/batched so the ACT table
    set switches only ~4 times total.
  - Weights are pre-folded with the rms weight vectors and cast to bf16 on
    host; matmuls run in bf16 (fp32 accumulation in PSUM).
"""

import sys

for _p in ("/opt/trn_rl_repo", "/root/.axon_site/_ro/trn_rl_repo"):
    if _p not in sys.path:
        sys.path.insert(0, _p)

from contextlib import ExitStack
from dataclasses import dataclass

import ml_dtypes
import numpy as np

import concourse.bass as bass
import concourse.tile as tile
from concourse import bacc, mybir
from concourse.masks import make_identity

F32 = mybir.dt.float32
BF16 = mybir.dt.bfloat16
AF = mybir.ActivationFunctionType
ALU = mybir.AluOpType

EPS = 1e-6


@dataclass(frozen=True)
class Cfg:
    S: int = 4096
    D: int = 1024
    F: int = 3072
    Tb: int = 256  # token block (matmul moving free dim)

    @property
    def NB(self):
        return self.S // self.Tb

    @property
    def TC(self):
        return self.Tb // 128  # token chunks per block

    @property
    def KD(self):
        return self.D // 128  # D in 128-chunks

    @property
    def KF(self):
        return self.F // 128  # F in 128-chunks


I32 = mybir.dt.int32


def _rsqrt_dve(nc, pool, ms, shape, tag, iters=3):
    """r = 1/sqrt(ms) entirely on the DVE: quake-III magic seed + Newton.

    Avoids ACT Sqrt so no activation-table switches are needed mid-kernel.
    ms is an f32 AP; returns an f32 tile of `shape`.
    """
    ti = pool.tile(shape, I32, tag=f"{tag}_i", name=f"{tag}_i")
    # 0x5f3759df - (i >> 1) == ((i >> 1) ^ -1) + 0x5f3759e0 in two's
    # complement; bitwise and arith ops can't share one tensor_scalar
    nc.vector.tensor_scalar(ti, ms.bitcast(I32), 1, -1,
                            op0=ALU.logical_shift_right, op1=ALU.bitwise_xor)
    nc.vector.tensor_scalar(ti, ti, 0x5F3759E0, None, op0=ALU.add)
    r = pool.tile(shape, F32, tag=f"{tag}_r", name=f"{tag}_r")
    nc.vector.tensor_copy(r, ti.bitcast(F32))
    t1 = pool.tile(shape, F32, tag=f"{tag}_t", name=f"{tag}_t")
    for _ in range(iters):
        nc.vector.tensor_mul(t1, r, r)
        nc.vector.tensor_mul(t1, t1, ms)
        nc.vector.tensor_scalar(t1, t1, -0.5, 1.5, op0=ALU.mult, op1=ALU.add)
        nc.vector.tensor_mul(r, r, t1)
    return r


def build_mingru(tc: tile.TileContext, outs: dict, ins: dict, cfg: Cfg):
    nc = tc.nc
    S, D, F_, Tb = cfg.S, cfg.D, cfg.F, cfg.Tb
    NB, TC, KD, KF = cfg.NB, cfg.TC, cfg.KD, cfg.KF

    x = ins["x"]  # [S, D] f32
    wg, wv, wd = ins["wg"], ins["wv"], ins["wd"]  # [D, D] bf16 (rms-folded)
    bg, bv, bd = ins["bg"], ins["bv"], ins["bd"]  # [KD, 128] f32
    wgate, wup = ins["wgate"], ins["wup"]  # [D, F] bf16 (rms-folded)
    wout = ins["wout"]  # [F, D] bf16
    out = outs["out"]  # [S, D] f32

    ctx = ExitStack()
    with ctx:
        singles = ctx.enter_context(tc.tile_pool(name="singles", bufs=1))
        dram = ctx.enter_context(tc.tile_pool(name="dram", bufs=1, space="DRAM"))

        ident = singles.tile([128, 128], F32)
        make_identity(nc, ident)
        ones_row = singles.tile([1, 128], F32)
        nc.gpsimd.memset(ones_row, 1.0)
        ones_col = singles.tile([128, 1], BF16)
        nc.gpsimd.memset(ones_col, 1.0)

        # biases as [128, KD] so bias[:, m] is a per-partition scalar AP
        bgs = singles.tile([128, KD], F32)
        bvs = singles.tile([128, KD], F32)
        bds = singles.tile([128, KD], F32)
        nc.sync.dma_start(out=bgs, in_=bg.rearrange("m p -> p m"))
        nc.sync.dma_start(out=bvs, in_=bv.rearrange("m p -> p m"))
        nc.sync.dma_start(out=bds, in_=bd.rearrange("m p -> p m"))

        # DRAM scratch
        x2t_d = dram.tile([D, S], F32)
        rms2_d = dram.tile([1, S], F32)
        hid_d = dram.tile([F_, S], BF16)

        # ---------------- phase 1: mixer ----------------
        prev_h = {}
        with tc.tile_pool(name="wmix", bufs=1) as wmix, tc.tile_pool(
            name="p1", bufs=2
        ) as p1, tc.tile_pool(name="p1h", bufs=2) as p1h, tc.tile_pool(
            name="ps_tr", bufs=2, space="PSUM"
        ) as ps_tr, tc.tile_pool(
            name="ps_gvd", bufs=1, space="PSUM"
        ) as ps_gvd, tc.tile_pool(
            name="ps_ss2", bufs=2, space="PSUM"
        ) as ps_ss2:
            # mixer weights resident: [128, D] bf16 per k-chunk
            wg_sb = [wmix.tile([128, D], BF16, tag=f"wg{k}", name=f"wg{k}") for k in range(KD)]
            wv_sb = [wmix.tile([128, D], BF16, tag=f"wv{k}", name=f"wv{k}") for k in range(KD)]
            wd_sb = [wmix.tile([128, D], BF16, tag=f"wd{k}", name=f"wd{k}") for k in range(KD)]
            for k in range(KD):
                nc.sync.dma_start(out=wg_sb[k], in_=wg[k * 128 : (k + 1) * 128, :])
                nc.sync.dma_start(out=wv_sb[k], in_=wv[k * 128 : (k + 1) * 128, :])
                nc.sync.dma_start(out=wd_sb[k], in_=wd[k * 128 : (k + 1) * 128, :])

            for j in range(NB):
                t0 = j * Tb
                # load x block as [p, c, d]
                xblk = p1.tile([128, TC, D], F32, tag="xblk", name="xblk")
                nc.sync.dma_start(
                    out=xblk,
                    in_=x[t0 : t0 + Tb, :].rearrange("(c p) d -> p c d", p=128),
                )

                # rms1 for this block: ACT square+accum -> DVE rsqrt
                ss1 = p1.tile([128, TC], F32, tag="ss1", name="ss1")
                for c in range(TC):
                    sqdump = p1.tile([128, D], BF16, tag="sqdump", name="sqdump")
                    nc.scalar.activation(
                        sqdump, xblk[:, c, :], AF.Square,
                        accum_out=ss1[:, c : c + 1],
                    )
                nc.vector.tensor_scalar(
                    ss1, ss1, 1.0 / D, EPS, op0=ALU.mult, op1=ALU.add
                )
                rms1 = _rsqrt_dve(nc, p1, ss1, [128, TC], "rms1")
                # transpose to a [1, Tb] row on partition 0, then broadcast
                # down 128 partitions via a K=1 ones matmul
                rowps = ps_ss2.tile([1, Tb], F32, tag="ss2", name="rowps")
                for c in range(TC):
                    nc.tensor.transpose(
                        rowps[0:1, c * 128 : (c + 1) * 128],
                        rms1[:, c : c + 1], ident,
                    )
                rms1row = p1.tile([1, Tb], F32, tag="rms1row", name="rms1row")
                nc.vector.tensor_copy(rms1row, rowps)
                rb = ps_tr.tile([128, Tb], F32, tag="rb", bufs=1, name="rb")
                for c in range(TC):
                    nc.tensor.matmul(
                        rb[:, c * 128 : (c + 1) * 128],
                        lhsT=ones_row,
                        rhs=rms1row[0:1, c * 128 : (c + 1) * 128],
                        start=True,
                        stop=True,
                    )

                # transpose x -> x^T tiles, and xn^T = x^T * rms1 (bf16)
                xT = []
                xnT = []
                for m in range(KD):
                    pt = ps_tr.tile([128, Tb], F32, tag="ptr", name="ptr")
                    for c in range(TC):
                        nc.tensor.transpose(
                            pt[:, c * 128 : (c + 1) * 128],
                            xblk[:, c, m * 128 : (m + 1) * 128],
                            ident,
                        )
                    xT_m = p1.tile([128, Tb], F32, tag=f"xT{m}", name=f"xT{m}")
                    nc.vector.tensor_copy(xT_m, pt)
                    xnT_m = p1.tile([128, Tb], BF16, tag=f"xnT{m}", name=f"xnT{m}")
                    nc.vector.tensor_mul(xnT_m, xT_m, rb)
                    xT.append(xT_m)
                    xnT.append(xnT_m)

                # mixer matmuls + epilogue + scan, per output d-chunk
                for m in range(KD):
                    psg = ps_gvd.tile([128, Tb], F32, tag="psg", name="psg")
                    psv = ps_gvd.tile([128, Tb], F32, tag="psv", name="psv")
                    psd = ps_gvd.tile([128, Tb], F32, tag="psd", name="psd")
                    for k in range(KD):
                        st, sp = (k == 0), (k == KD - 1)
                        nc.tensor.matmul(
                            psg, lhsT=wg_sb[k][:, m * 128 : (m + 1) * 128],
                            rhs=xnT[k], start=st, stop=sp,
                        )
                        nc.tensor.matmul(
                            psv, lhsT=wv_sb[k][:, m * 128 : (m + 1) * 128],
                            rhs=xnT[k], start=st, stop=sp,
                        )
                        nc.tensor.matmul(
                            psd, lhsT=wd_sb[k][:, m * 128 : (m + 1) * 128],
                            rhs=xnT[k], start=st, stop=sp,
                        )
                    sg = p1.tile([128, Tb], F32, tag="sg", name="sg")
                    nc.scalar.activation(sg, psg, AF.Sigmoid, bias=bgs[:, m : m + 1])
                    tv = p1.tile([128, Tb], F32, tag="tv", name="tv")
                    nc.scalar.activation(tv, psv, AF.Tanh, bias=bvs[:, m : m + 1])
                    sd = p1.tile([128, Tb], F32, tag="sd", name="sd")
                    nc.scalar.activation(sd, psd, AF.Sigmoid, bias=bds[:, m : m + 1])

                    xs = p1.tile([128, Tb], F32, tag="xs", name="xs")
                    nc.vector.tensor_mul(xs, sg, tv)
                    aa = p1.tile([128, Tb], F32, tag="aa", name="aa")
                    nc.vector.tensor_scalar(
                        aa, sd, 0.998, 0.001, op0=ALU.mult, op1=ALU.add
                    )

                    h_m = p1h.tile([128, Tb], F32, tag=f"h{m}", name=f"h{m}")
                    init = 0.0 if j == 0 else prev_h[m][:, Tb - 1 : Tb]
                    nc.vector.tensor_tensor_scan(
                        h_m, data0=aa, data1=xs, initial=init,
                        op0=ALU.mult, op1=ALU.add,
                    )
                    prev_h[m] = h_m

                    x2T_m = p1.tile([128, Tb], F32, tag=f"x2T{m}", name=f"x2T{m}")
                    nc.vector.tensor_add(x2T_m, xT[m], h_m)
                    nc.sync.dma_start(
                        out=x2t_d[m * 128 : (m + 1) * 128, t0 : t0 + Tb],
                        in_=x2T_m,
                    )
                    # FFN-norm sum of squares: ones^T @ x2^2 accumulated over m
                    sq2 = p1.tile([128, Tb], BF16, tag="sq2", name="sq2")
                    nc.scalar.activation(sq2, x2T_m, AF.Square)
                    ss2_ps = (
                        ps_ss2.tile([1, Tb], F32, tag="ss2", name="ss2")
                        if m == 0
                        else ss2_ps
                    )
                    nc.tensor.matmul(
                        ss2_ps, lhsT=ones_col, rhs=sq2,
                        start=(m == 0), stop=(m == KD - 1),
                    )
                # rms2 for this block -> DRAM row (used by phase 2a)
                ss2_sb = p1.tile([1, Tb], F32, tag="ss2sb", name="ss2sb")
                nc.vector.tensor_copy(ss2_sb, ss2_ps)
                nc.vector.tensor_scalar(
                    ss2_sb, ss2_sb, 1.0 / D, EPS, op0=ALU.mult, op1=ALU.add
                )
                rms2 = _rsqrt_dve(nc, p1, ss2_sb, [1, Tb], "rms2")
                nc.sync.dma_start(out=rms2_d[:, t0 : t0 + Tb], in_=rms2)

        # ---------------- phase 2a: gate/up -> hidden ----------------
        with tc.tile_pool(name="wglu", bufs=1) as wglu, tc.tile_pool(
            name="p2a", bufs=2
        ) as p2a, tc.tile_pool(name="ps_2a", bufs=2, space="PSUM") as ps_2a:
            wgate_sb = [wglu.tile([128, F_], BF16, tag=f"wgate{k}", name=f"wgate{k}") for k in range(KD)]
            wup_sb = [wglu.tile([128, F_], BF16, tag=f"wup{k}", name=f"wup{k}") for k in range(KD)]
            for k in range(KD):
                nc.sync.dma_start(out=wgate_sb[k], in_=wgate[k * 128 : (k + 1) * 128, :])
                nc.sync.dma_start(out=wup_sb[k], in_=wup[k * 128 : (k + 1) * 128, :])

            for j in range(NB):
                t0 = j * Tb
                rms2row = p2a.tile([1, Tb], F32, tag="rms2row", name="rms2row")
                nc.sync.dma_start(out=rms2row, in_=rms2_d[:, t0 : t0 + Tb])
                rb2 = ps_2a.tile([128, Tb], F32, tag="rb2", name="rb2")
                for c in range(TC):
                    nc.tensor.matmul(
                        rb2[:, c * 128 : (c + 1) * 128],
                        lhsT=ones_row,
                        rhs=rms2row[0:1, c * 128 : (c + 1) * 128],
                        start=True,
                        stop=True,
                    )
                x2nT = []
                for m in range(KD):
                    x2a_m = p2a.tile([128, Tb], F32, tag=f"x2a{m}", name=f"x2a{m}")
                    nc.sync.dma_start(
                        out=x2a_m,
                        in_=x2t_d[m * 128 : (m + 1) * 128, t0 : t0 + Tb],
                    )
                    x2nT_m = p2a.tile([128, Tb], BF16, tag=f"x2nT{m}", name=f"x2nT{m}")
                    nc.vector.tensor_mul(x2nT_m, x2a_m, rb2)
                    x2nT.append(x2nT_m)

                for f in range(KF):
                    pg = ps_2a.tile([128, Tb], F32, tag="pg", name="pg")
                    pu = ps_2a.tile([128, Tb], F32, tag="pu", name="pu")
                    for k in range(KD):
                        st, sp = (k == 0), (k == KD - 1)
                        nc.tensor.matmul(
                            pg, lhsT=wgate_sb[k][:, f * 128 : (f + 1) * 128],
                            rhs=x2nT[k], start=st, stop=sp,
                        )
                        nc.tensor.matmul(
                            pu, lhsT=wup_sb[k][:, f * 128 : (f + 1) * 128],
                            rhs=x2nT[k], start=st, stop=sp,
                        )
                    # silu(g) = g * sigmoid(g), composed so each DVE op reads
                    # at most one PSUM operand
                    sl = p2a.tile([128, Tb], F32, tag="sl", name="sl")
                    nc.scalar.activation(sl, pg, AF.Sigmoid)
                    sl2 = p2a.tile([128, Tb], F32, tag="sl2", name="sl2")
                    nc.vector.tensor_mul(sl2, sl, pg)
                    hid = p2a.tile([128, Tb], BF16, tag="hid", bufs=3, name="hid")
                    nc.vector.tensor_mul(hid, sl2, pu)
                    nc.sync.dma_start(
                        out=hid_d[f * 128 : (f + 1) * 128, t0 : t0 + Tb], in_=hid
                    )

        # ---------------- phase 2b: out matmul + residual + transpose ----------------
        with tc.tile_pool(name="wo", bufs=1) as wo, tc.tile_pool(
            name="p2b", bufs=2
        ) as p2b, tc.tile_pool(name="ps_2b", bufs=2, space="PSUM") as ps_2b:
            wout_sb = [wo.tile([128, D], BF16, tag=f"wout{k}", name=f"wout{k}") for k in range(KF)]
            for k in range(KF):
                nc.sync.dma_start(out=wout_sb[k], in_=wout[k * 128 : (k + 1) * 128, :])

            for j in range(NB):
                t0 = j * Tb
                hidT = []
                for k in range(KF):
                    hT_k = p2b.tile([128, Tb], BF16, tag=f"hT{k}", name=f"hT{k}")
                    nc.sync.dma_start(
                        out=hT_k, in_=hid_d[k * 128 : (k + 1) * 128, t0 : t0 + Tb]
                    )
                    hidT.append(hT_k)
                outT = []
                for m in range(KD):
                    x2b_m = p2b.tile([128, Tb], F32, tag=f"x2b{m}", name=f"x2b{m}")
                    nc.sync.dma_start(
                        out=x2b_m,
                        in_=x2t_d[m * 128 : (m + 1) * 128, t0 : t0 + Tb],
                    )
                    pf = ps_2b.tile([128, Tb], F32, tag="pf", name="pf")
                    for k in range(KF):
                        nc.tensor.matmul(
                            pf, lhsT=wout_sb[k][:, m * 128 : (m + 1) * 128],
                            rhs=hidT[k], start=(k == 0), stop=(k == KF - 1),
                        )
                    outT_m = p2b.tile([128, Tb], F32, tag=f"outT{m}", name=f"outT{m}")
                    nc.vector.tensor_add(outT_m, x2b_m, pf)
                    outT.append(outT_m)
                # transpose back to natural [token, D] and store
                for c in range(TC):
                    obl = p2b.tile([128, D], F32, tag="obl", bufs=3, name="obl")
                    for m in range(KD):
                        pt2 = ps_2b.tile([128, 128], F32, tag="pt2", name="pt2")
                        nc.tensor.transpose(
                            pt2, outT[m][:, c * 128 : (c + 1) * 128], ident
                        )
                        nc.vector.tensor_copy(obl[:, m * 128 : (m + 1) * 128], pt2)
                    tt = t0 + c * 128
                    nc.sync.dma_start(out=out[tt : tt + 128, :], in_=obl)


# ----------------------------------------------------------------------------
# host side
# ----------------------------------------------------------------------------

def prep_weights(inputs: dict, cfg: Cfg):
    """Fold rms weight vectors into the matmul weights, cast to bf16, and
    reshape biases. Returns the per-core common input dict (everything except
    x)."""
    bf = ml_dtypes.bfloat16
    w_mix = np.asarray(inputs["w_rms_mix"], np.float32)[:, None]
    w_ffn = np.asarray(inputs["w_rms_ffn"], np.float32)[:, None]
    KD = cfg.D // 128
    return {
        "wg": (w_mix * np.asarray(inputs["Wg"], np.float32)).astype(bf),
        "wv": (w_mix * np.asarray(inputs["Wv"], np.float32)).astype(bf),
        "wd": (w_mix * np.asarray(inputs["Wd"], np.float32)).astype(bf),
        "bg": np.ascontiguousarray(
            np.asarray(inputs["bg"], np.float32).reshape(KD, 128)
        ),
        "bv": np.ascontiguousarray(
            np.asarray(inputs["bv"], np.float32).reshape(KD, 128)
        ),
        "bd": np.ascontiguousarray(
            np.asarray(inputs["bd"], np.float32).reshape(KD, 128)
        ),
        "wgate": (w_ffn * np.asarray(inputs["W_gate"], np.float32)).astype(bf),
        "wup": (w_ffn * np.asarray(inputs["W_up"], np.float32)).astype(bf),
        "wout": np.asarray(inputs["W_out"], np.float32).astype(bf),
    }


def build_nc(cfg: Cfg):
    bf = mybir.dt.bfloat16
    # Bacc (not bare Bass): its compile() pass splits multi-wait sync into
    # event semaphores (HW allows at most 1 wait per instruction) and
    # hoists ACT table loads.
    nc = bacc.Bacc("TRN2", target_bir_lowering=False, debug=False)
    KD = cfg.D // 128
    ins = {
        "x": nc.declare_dram_parameter("x", [cfg.S, cfg.D], F32, isOutput=False),
        "wg": nc.declare_dram_parameter("wg", [cfg.D, cfg.D], bf, isOutput=False),
        "wv": nc.declare_dram_parameter("wv", [cfg.D, cfg.D], bf, isOutput=False),
        "wd": nc.declare_dram_parameter("wd", [cfg.D, cfg.D], bf, isOutput=False),
        "bg": nc.declare_dram_parameter("bg", [KD, 128], F32, isOutput=False),
        "bv": nc.declare_dram_parameter("bv", [KD, 128], F32, isOutput=False),
        "bd": nc.declare_dram_parameter("bd", [KD, 128], F32, isOutput=False),
        "wgate": nc.declare_dram_parameter("wgate", [cfg.D, cfg.F], bf, isOutput=False),
        "wup": nc.declare_dram_parameter("wup", [cfg.D, cfg.F], bf, isOutput=False),
        "wout": nc.declare_dram_parameter("wout", [cfg.F, cfg.D], bf, isOutput=False),
    }
    outs = {
        "out": nc.declare_dram_parameter("out", [cfg.S, cfg.D], F32, isOutput=True),
    }
    ins_ap = {k: v.ap() for k, v in ins.items()}
    outs_ap = {k: v.ap() for k, v in outs.items()}
    with tile.TileContext(nc, pool_alloc_mode="queue") as tc:
        build_mingru(tc, outs_ap, ins_ap, cfg)
    nc.compile()
    return nc


_NC_CACHE = {}


def kernel(**inputs) -> np.ndarray:
    from concourse.bass_utils import run_bass_kernel_spmd

    cfg = Cfg()
    x = np.asarray(inputs["x"], np.float32)  # [B, S, D]
    B = x.shape[0]
    common = prep_weights(inputs, cfg)

    if cfg not in _NC_CACHE:
        _NC_CACHE[cfg] = build_nc(cfg)
    nc = _NC_CACHE[cfg]

    in_maps = [dict(common, x=np.ascontiguousarray(x[b])) for b in range(B)]
    res = run_bass_kernel_spmd(nc, in_maps, core_ids=list(range(B)))
    out = np.stack([np.asarray(res.results[b]["out"]) for b in range(B)], axis=0)
    return out.astype(np.float32)


def _ensure_ntff_hook():
    """Register the axon NTFF profile hook if the agent image's antenv lacks
    axon_hooks (same ctypes shim trn_boot would install)."""
    import contextlib
    import ctypes
    import types

    try:
        from antenv.axon_hooks import get_axon_ntff_profile_hook

        if get_axon_ntff_profile_hook() is not None:
            return
    except ImportError:
        pass

    so_path = "/opt/axon/libaxon_pjrt.so"
    lib = ctypes.CDLL(so_path)
    if not hasattr(lib, "axon_start_nrt_profile"):
        return
    lib.axon_start_nrt_profile.argtypes = [
        ctypes.POINTER(ctypes.c_int64),
        ctypes.c_size_t,
    ]
    lib.axon_start_nrt_profile.restype = ctypes.c_int64
    lib.axon_stop_nrt_profile.argtypes = [ctypes.c_char_p]
    lib.axon_stop_nrt_profile.restype = ctypes.c_int64

    @contextlib.contextmanager
    def _hook(output_dir, device_ids):
        import jax

        jax.devices()
        if device_ids:
            ids = (ctypes.c_int64 * len(device_ids))(*device_ids)
            rc = lib.axon_start_nrt_profile(ids, len(device_ids))
        else:
            rc = lib.axon_start_nrt_profile(None, 0)
        if rc != 0:
            raise RuntimeError(f"axon_start_nrt_profile rc={rc}")
        try:
            yield
        finally:
            n = lib.axon_stop_nrt_profile(str(output_dir).encode())
            print(f"profile: {n} file(s) written to {output_dir}")

    mod = types.ModuleType("antenv.axon_hooks")
    mod.get_axon_ntff_profile_hook = lambda: _hook
    mod.set_axon_ntff_profile_hook = lambda h: None
    sys.modules["antenv.axon_hooks"] = mod
    import antenv

    antenv.axon_hooks = mod


def kernel_profiled(**inputs):
    """Run once with NTFF tracing; returns exec_time_ns (max across cores)."""
    from concourse import bass_utils
    from concourse.bass_utils import run_bass_kernel_spmd

    _ensure_ntff_hook()
    # skip the bucket upload (no creds needed for local analysis)
    bass_utils.upload_artifacts = lambda tmpdir: f"local:{tmpdir}"

    cfg = Cfg()
    x = np.asarray(inputs["x"], np.float32)
    B = x.shape[0]
    common = prep_weights(inputs, cfg)
    if cfg not in _NC_CACHE:
        _NC_CACHE[cfg] = build_nc(cfg)
    nc = _NC_CACHE[cfg]
    in_maps = [dict(common, x=np.ascontiguousarray(x[b])) for b in range(B)]
    import os
    tmpdir = "/tmp/mingru_profile"
    os.makedirs(tmpdir, exist_ok=True)
    res = run_bass_kernel_spmd(
        nc, in_maps, core_ids=list(range(B)), trace=True, tmpdir=tmpdir
    )
    return res.exec_time_ns


if __name__ == "__main__":
    rng = np.random.default_rng(0)
    cfg = Cfg()
    fake = {
        "x": rng.standard_normal((8, cfg.S, cfg.D), dtype=np.float32),
        "w_rms_mix": np.ones(cfg.D, np.float32),
        "w_rms_ffn": np.ones(cfg.D, np.float32),
        "Wg": rng.standard_normal((cfg.D, cfg.D), dtype=np.float32) / 32,
        "bg": np.zeros(cfg.D, np.float32),
        "Wv": rng.standard_normal((cfg.D, cfg.D), dtype=np.float32) / 32,
        "bv": np.zeros(cfg.D, np.float32),
        "Wd": rng.standard_normal((cfg.D, cfg.D), dtype=np.float32) / 32,
        "bd": np.ones(cfg.D, np.float32),
        "W_gate": rng.standard_normal((cfg.D, cfg.F), dtype=np.float32) / 32,
        "W_up": rng.standard_normal((cfg.D, cfg.F), dtype=np.float32) / 32,
        "W_out": rng.standard_normal((cfg.F, cfg.D), dtype=np.float32) / 55,
    }
    out = kernel(**fake)
    print(out.shape, out.dtype)


# revision 25
# speedup vs baseline: 1.0854x; 1.0322x over previous
"""MinGRU block kernel for Trainium2 (Bass/Tile), 8-core data-parallel over batch.

Reference computation (per batch b):
    xn = rmsnorm(x, w_rms_mix)
    g = xn@Wg+bg; v = xn@Wv+bv; d = xn@Wd+bd
    x_scan = sigmoid(g)*tanh(v);  a = 0.001 + 0.998*sigmoid(d)
    h = linear_scan(x_scan, a)          # h_t = a_t h_{t-1} + x_t along S
    x2 = x + h
    yn = rmsnorm(x2, w_rms_ffn)
    out = x2 + (silu(yn@W_gate) * (yn@W_up)) @ W_out

Shapes: B=8, S=4096, D=1024, F=3072 (fp32).  Each core handles one batch.

Design notes:
  - All matmul activations live in transposed layout [feature, token] so the
    contraction dim (features) is on partitions; weights are used directly as
    lhsT in their natural [in, out] storage.  The sequential scan runs along
    the free (token) axis via the DVE tensor_tensor_scan instruction.
  - Phase 1 (mixer): transpose x (PE), g/v/d matmuls, sigmoid/tanh epilogue,
    scan with carried state across token blocks, x2^T = x^T + h^T, plus the
    FFN-norm sum-of-squares (ones-matmul over partitions).  x2^T spills to
    DRAM.
  - Phase 2a: gate/up matmuls from x2n^T, silu*up -> hidden^T (spills).
  - Phase 2b: out matmul, residual add, PE transpose-back to natural layout.
  - rsqrt for rmsnorm: ACT Sqrt + DVE reciprocal + one Newton step (ACT Rsqrt
    is banned for accuracy).  Sqrt calls are hoisted/batched so the ACT table
    set switches only ~4 times total.
  - Weights are pre-folded with the rms weight vectors and cast to bf16 on
    host; matmuls run in bf16 (fp32 accumulation in PSUM).
"""

import sys

for _p in ("/opt/trn_rl_repo", "/root/.axon_site/_ro/trn_rl_repo"):
    if _p not in sys.path:
        sys.path.insert(0, _p)

from contextlib import ExitStack
from dataclasses import dataclass

import ml_dtypes
import numpy as np

import concourse.bass as bass
import concourse.tile as tile
from concourse import bacc, mybir
from concourse.masks import make_identity

F32 = mybir.dt.float32
BF16 = mybir.dt.bfloat16
AF = mybir.ActivationFunctionType
ALU = mybir.AluOpType

EPS = 1e-6


@dataclass(frozen=True)
class Cfg:
    S: int = 4096
    D: int = 1024
    F: int = 3072
    Tb: int = 256  # token block (matmul moving free dim)

    @property
    def NB(self):
        return self.S // self.Tb

    @property
    def TC(self):
        return self.Tb // 128  # token chunks per block

    @property
    def KD(self):
        return self.D // 128  # D in 128-chunks

    @property
    def KF(self):
        return self.F // 128  # F in 128-chunks


I32 = mybir.dt.int32


def _rsqrt_dve(nc, pool, ms, shape, tag, iters=3):
    """r = 1/sqrt(ms) entirely on the DVE: quake-III magic seed + Newton.

    Avoids ACT Sqrt so no activation-table switches are needed mid-kernel.
    ms is an f32 AP; returns an f32 tile of `shape`.
    """
    ti = pool.tile(shape, I32, tag=f"{tag}_i", name=f"{tag}_i")
    # 0x5f3759df - (i >> 1) == ((i >> 1) ^ -1) + 0x5f3759e0 in two's
    # complement; bitwise and arith ops can't share one tensor_scalar
    nc.vector.tensor_scalar(ti, ms.bitcast(I32), 1, -1,
                            op0=ALU.logical_shift_right, op1=ALU.bitwise_xor)
    nc.vector.tensor_scalar(ti, ti, 0x5F3759E0, None, op0=ALU.add)
    r = pool.tile(shape, F32, tag=f"{tag}_r", name=f"{tag}_r")
    nc.vector.tensor_copy(r, ti.bitcast(F32))
    t1 = pool.tile(shape, F32, tag=f"{tag}_t", name=f"{tag}_t")
    for _ in range(iters):
        nc.vector.tensor_mul(t1, r, r)
        nc.vector.tensor_mul(t1, t1, ms)
        nc.vector.tensor_scalar(t1, t1, -0.5, 1.5, op0=ALU.mult, op1=ALU.add)
        nc.vector.tensor_mul(r, r, t1)
    return r


def build_mingru(tc: tile.TileContext, outs: dict, ins: dict, cfg: Cfg):
    nc = tc.nc
    S, D, F_, Tb = cfg.S, cfg.D, cfg.F, cfg.Tb
    NB, TC, KD, KF = cfg.NB, cfg.TC, cfg.KD, cfg.KF

    x = ins["x"]  # [S, D] f32
    wg, wv, wd = ins["wg"], ins["wv"], ins["wd"]  # [D, D] bf16 (rms-folded)
    bg, bv, bd = ins["bg"], ins["bv"], ins["bd"]  # [KD, 128] f32
    wgate, wup = ins["wgate"], ins["wup"]  # [D, F] bf16 (rms-folded)
    wout = ins["wout"]  # [F, D] bf16
    out = outs["out"]  # [S, D] f32

    ctx = ExitStack()
    with ctx:
        singles = ctx.enter_context(tc.tile_pool(name="singles", bufs=1))
        dram = ctx.enter_context(tc.tile_pool(name="dram", bufs=1, space="DRAM"))

        ident = singles.tile([128, 128], F32)
        make_identity(nc, ident)
        ones_row = singles.tile([1, 128], F32)
        nc.gpsimd.memset(ones_row, 1.0)
        ones_col = singles.tile([128, 1], BF16)
        nc.gpsimd.memset(ones_col, 1.0)

        # biases as [128, KD] so bias[:, m] is a per-partition scalar AP
        bgs = singles.tile([128, KD], F32)
        bvs = singles.tile([128, KD], F32)
        bds = singles.tile([128, KD], F32)
        nc.sync.dma_start(out=bgs, in_=bg.rearrange("m p -> p m"))
        nc.sync.dma_start(out=bvs, in_=bv.rearrange("m p -> p m"))
        nc.sync.dma_start(out=bds, in_=bd.rearrange("m p -> p m"))

        # DRAM scratch
        x2t_d = dram.tile([D, S], F32)
        rms2_d = dram.tile([1, S], F32)

        # ---------------- phase 1: mixer ----------------
        prev_h = {}
        with tc.tile_pool(name="wmix", bufs=1) as wmix, tc.tile_pool(
            name="p1", bufs=2
        ) as p1, tc.tile_pool(name="p1h", bufs=2) as p1h, tc.tile_pool(
            name="ps_tr", bufs=2, space="PSUM"
        ) as ps_tr, tc.tile_pool(
            name="ps_gvd", bufs=1, space="PSUM"
        ) as ps_gvd, tc.tile_pool(
            name="ps_ss2", bufs=2, space="PSUM"
        ) as ps_ss2:
            # first x block loads before the weights so the transpose/norm
            # pipeline fills while weights stream in
            xblk0 = p1.tile([128, TC, D], F32, tag="xblk", name="xblk0")
            nc.sync.dma_start(
                out=xblk0, in_=x[0:Tb, :].rearrange("(c p) d -> p c d", p=128)
            )

            # mixer weights resident: [128, D] bf16 per k-chunk
            wg_sb = [wmix.tile([128, D], BF16, tag=f"wg{k}", name=f"wg{k}") for k in range(KD)]
            wv_sb = [wmix.tile([128, D], BF16, tag=f"wv{k}", name=f"wv{k}") for k in range(KD)]
            wd_sb = [wmix.tile([128, D], BF16, tag=f"wd{k}", name=f"wd{k}") for k in range(KD)]
            for k in range(KD):
                nc.sync.dma_start(out=wg_sb[k], in_=wg[k * 128 : (k + 1) * 128, :])
                nc.sync.dma_start(out=wv_sb[k], in_=wv[k * 128 : (k + 1) * 128, :])
                nc.sync.dma_start(out=wd_sb[k], in_=wd[k * 128 : (k + 1) * 128, :])

            for j in range(NB):
                t0 = j * Tb
                # load x block as [p, c, d]
                if j == 0:
                    xblk = xblk0
                else:
                    xblk = p1.tile([128, TC, D], F32, tag="xblk", name="xblk")
                    nc.sync.dma_start(
                        out=xblk,
                        in_=x[t0 : t0 + Tb, :].rearrange("(c p) d -> p c d", p=128),
                    )

                # rms1 for this block: ACT square+accum -> DVE rsqrt
                ss1 = p1.tile([128, TC], F32, tag="ss1", name="ss1")
                for c in range(TC):
                    sqdump = p1.tile([128, D], BF16, tag="sqdump", name="sqdump")
                    nc.scalar.activation(
                        sqdump, xblk[:, c, :], AF.Square,
                        accum_out=ss1[:, c : c + 1],
                    )
                nc.vector.tensor_scalar(
                    ss1, ss1, 1.0 / D, EPS, op0=ALU.mult, op1=ALU.add
                )
                rms1 = _rsqrt_dve(nc, p1, ss1, [128, TC], "rms1")
                # transpose to a [1, Tb] row on partition 0, then broadcast
                # down 128 partitions via a K=1 ones matmul
                rowps = ps_ss2.tile([1, Tb], F32, tag="ss2", name="rowps")
                for c in range(TC):
                    nc.tensor.transpose(
                        rowps[0:1, c * 128 : (c + 1) * 128],
                        rms1[:, c : c + 1], ident,
                    )
                rms1row = p1.tile([1, Tb], F32, tag="rms1row", name="rms1row")
                nc.vector.tensor_copy(rms1row, rowps)
                rb = ps_tr.tile([128, Tb], F32, tag="rb", bufs=1, name="rb")
                for c in range(TC):
                    nc.tensor.matmul(
                        rb[:, c * 128 : (c + 1) * 128],
                        lhsT=ones_row,
                        rhs=rms1row[0:1, c * 128 : (c + 1) * 128],
                        start=True,
                        stop=True,
                    )

                # transpose x -> x^T tiles, and xn^T = x^T * rms1 (bf16)
                xT = []
                xnT = []
                for m in range(KD):
                    pt = ps_tr.tile([128, Tb], F32, tag="ptr", name="ptr")
                    for c in range(TC):
                        nc.tensor.transpose(
                            pt[:, c * 128 : (c + 1) * 128],
                            xblk[:, c, m * 128 : (m + 1) * 128],
                            ident,
                        )
                    xT_m = p1.tile([128, Tb], F32, tag=f"xT{m}", name=f"xT{m}")
                    nc.vector.tensor_copy(xT_m, pt)
                    xnT_m = p1.tile([128, Tb], BF16, tag=f"xnT{m}", name=f"xnT{m}")
                    nc.vector.tensor_mul(xnT_m, xT_m, rb)
                    xT.append(xT_m)
                    xnT.append(xnT_m)

                # mixer matmuls + epilogue + scan, per output d-chunk
                for m in range(KD):
                    psg = ps_gvd.tile([128, Tb], F32, tag="psg", name="psg")
                    psv = ps_gvd.tile([128, Tb], F32, tag="psv", name="psv")
                    psd = ps_gvd.tile([128, Tb], F32, tag="psd", name="psd")
                    for k in range(KD):
                        st, sp = (k == 0), (k == KD - 1)
                        nc.tensor.matmul(
                            psg, lhsT=wg_sb[k][:, m * 128 : (m + 1) * 128],
                            rhs=xnT[k], start=st, stop=sp,
                        )
                        nc.tensor.matmul(
                            psv, lhsT=wv_sb[k][:, m * 128 : (m + 1) * 128],
                            rhs=xnT[k], start=st, stop=sp,
                        )
                        nc.tensor.matmul(
                            psd, lhsT=wd_sb[k][:, m * 128 : (m + 1) * 128],
                            rhs=xnT[k], start=st, stop=sp,
                        )
                    sg = p1.tile([128, Tb], F32, tag="sg", name="sg")
                    nc.scalar.activation(sg, psg, AF.Sigmoid, bias=bgs[:, m : m + 1])
                    tv = p1.tile([128, Tb], F32, tag="tv", name="tv")
                    nc.scalar.activation(tv, psv, AF.Tanh, bias=bvs[:, m : m + 1])
                    sd = p1.tile([128, Tb], F32, tag="sd", name="sd")
                    nc.scalar.activation(sd, psd, AF.Sigmoid, bias=bds[:, m : m + 1])

                    xs = p1.tile([128, Tb], F32, tag="xs", name="xs")
                    nc.vector.tensor_mul(xs, sg, tv)
                    aa = p1.tile([128, Tb], F32, tag="aa", name="aa")
                    nc.vector.tensor_scalar(
                        aa, sd, 0.998, 0.001, op0=ALU.mult, op1=ALU.add
                    )

                    h_m = p1h.tile([128, Tb], F32, tag=f"h{m}", name=f"h{m}")
                    init = 0.0 if j == 0 else prev_h[m][:, Tb - 1 : Tb]
                    nc.vector.tensor_tensor_scan(
                        h_m, data0=aa, data1=xs, initial=init,
                        op0=ALU.mult, op1=ALU.add,
                    )
                    prev_h[m] = h_m

                    x2T_m = p1.tile([128, Tb], F32, tag=f"x2T{m}", name=f"x2T{m}")
                    nc.vector.tensor_add(x2T_m, xT[m], h_m)
                    nc.sync.dma_start(
                        out=x2t_d[m * 128 : (m + 1) * 128, t0 : t0 + Tb],
                        in_=x2T_m,
                    )
                    # FFN-norm sum of squares: ones^T @ x2^2 accumulated over m
                    sq2 = p1.tile([128, Tb], BF16, tag="sq2", name="sq2")
                    nc.scalar.activation(sq2, x2T_m, AF.Square)
                    ss2_ps = (
                        ps_ss2.tile([1, Tb], F32, tag="ss2", name="ss2")
                        if m == 0
                        else ss2_ps
                    )
                    nc.tensor.matmul(
                        ss2_ps, lhsT=ones_col, rhs=sq2,
                        start=(m == 0), stop=(m == KD - 1),
                    )
                # rms2 for this block -> DRAM row (used by phase 2a)
                ss2_sb = p1.tile([1, Tb], F32, tag="ss2sb", name="ss2sb")
                nc.vector.tensor_copy(ss2_sb, ss2_ps)
                nc.vector.tensor_scalar(
                    ss2_sb, ss2_sb, 1.0 / D, EPS, op0=ALU.mult, op1=ALU.add
                )
                rms2 = _rsqrt_dve(nc, p1, ss2_sb, [1, Tb], "rms2")
                nc.sync.dma_start(out=rms2_d[:, t0 : t0 + Tb], in_=rms2)

        # ---------------- phase 2: full FFN (gate/up -> hidden -> out) --------
        # hidden stays in SBUF per block (no DRAM spill); x2^T is read once
        # per block and reused for both the norm input and the residual.
        with tc.tile_pool(name="wffn", bufs=1) as wffn, tc.tile_pool(
            name="p2", bufs=2
        ) as p2, tc.tile_pool(name="ps_2", bufs=2, space="PSUM") as ps_2:
            wgate_sb = [wffn.tile([128, F_], BF16, tag=f"wgate{k}", name=f"wgate{k}") for k in range(KD)]
            wup_sb = [wffn.tile([128, F_], BF16, tag=f"wup{k}", name=f"wup{k}") for k in range(KD)]
            wout_sb = [wffn.tile([128, D], BF16, tag=f"wout{k}", name=f"wout{k}") for k in range(KF)]
            # gate/up weights first (they gate the first matmuls); wout's
            # load hides behind the first block's gate/up compute
            for k in range(KD):
                nc.sync.dma_start(out=wgate_sb[k], in_=wgate[k * 128 : (k + 1) * 128, :])
                nc.sync.dma_start(out=wup_sb[k], in_=wup[k * 128 : (k + 1) * 128, :])
            for k in range(KF):
                nc.sync.dma_start(out=wout_sb[k], in_=wout[k * 128 : (k + 1) * 128, :])

            for j in range(NB):
                t0 = j * Tb
                rms2row = p2.tile([1, Tb], F32, tag="rms2row", name="rms2row")
                nc.sync.dma_start(out=rms2row, in_=rms2_d[:, t0 : t0 + Tb])
                rb2 = ps_2.tile([128, Tb], F32, tag="rb2", bufs=1, name="rb2")
                for c in range(TC):
                    nc.tensor.matmul(
                        rb2[:, c * 128 : (c + 1) * 128],
                        lhsT=ones_row,
                        rhs=rms2row[0:1, c * 128 : (c + 1) * 128],
                        start=True,
                        stop=True,
                    )
                x2a = []
                x2nT = []
                for m in range(KD):
                    x2a_m = p2.tile([128, Tb], F32, tag=f"x2a{m}", name=f"x2a{m}")
                    nc.sync.dma_start(
                        out=x2a_m,
                        in_=x2t_d[m * 128 : (m + 1) * 128, t0 : t0 + Tb],
                    )
                    x2nT_m = p2.tile([128, Tb], BF16, tag=f"x2nT{m}", name=f"x2nT{m}")
                    nc.vector.tensor_mul(x2nT_m, x2a_m, rb2)
                    x2a.append(x2a_m)
                    x2nT.append(x2nT_m)

                hidden = []
                for f in range(KF):
                    pg = ps_2.tile([128, Tb], F32, tag="pg", name="pg")
                    pu = ps_2.tile([128, Tb], F32, tag="pu", name="pu")
                    for k in range(KD):
                        st, sp = (k == 0), (k == KD - 1)
                        nc.tensor.matmul(
                            pg, lhsT=wgate_sb[k][:, f * 128 : (f + 1) * 128],
                            rhs=x2nT[k], start=st, stop=sp,
                        )
                        nc.tensor.matmul(
                            pu, lhsT=wup_sb[k][:, f * 128 : (f + 1) * 128],
                            rhs=x2nT[k], start=st, stop=sp,
                        )
                    # silu(g) = g * sigmoid(g), composed so each DVE op reads
                    # at most one PSUM operand
                    sl = p2.tile([128, Tb], F32, tag="sl", name="sl")
                    nc.scalar.activation(sl, pg, AF.Sigmoid)
                    sl2 = p2.tile([128, Tb], F32, tag="sl2", name="sl2")
                    nc.vector.tensor_mul(sl2, sl, pg)
                    hid = p2.tile([128, Tb], BF16, tag=f"hid{f}", bufs=1, name=f"hid{f}")
                    nc.vector.tensor_mul(hid, sl2, pu)
                    hidden.append(hid)

                outT = []
                for m in range(KD):
                    pf = ps_2.tile([128, Tb], F32, tag="pf", name="pf")
                    for k in range(KF):
                        nc.tensor.matmul(
                            pf, lhsT=wout_sb[k][:, m * 128 : (m + 1) * 128],
                            rhs=hidden[k], start=(k == 0), stop=(k == KF - 1),
                        )
                    outT_m = p2.tile([128, Tb], F32, tag=f"outT{m}", bufs=1, name=f"outT{m}")
                    nc.vector.tensor_add(outT_m, x2a[m], pf)
                    outT.append(outT_m)
                # transpose back to natural [token, D] and store
                for c in range(TC):
                    obl = p2.tile([128, D], F32, tag="obl", bufs=3, name="obl")
                    for m in range(KD):
                        pt2 = ps_2.tile([128, 128], F32, tag="pt2", bufs=1, name="pt2")
                        nc.tensor.transpose(
                            pt2, outT[m][:, c * 128 : (c + 1) * 128], ident
                        )
                        nc.vector.tensor_copy(obl[:, m * 128 : (m + 1) * 128], pt2)
                    tt = t0 + c * 128
                    nc.sync.dma_start(out=out[tt : tt + 128, :], in_=obl)


# ----------------------------------------------------------------------------
# host side
# ----------------------------------------------------------------------------

def prep_weights(inputs: dict, cfg: Cfg):
    """Fold rms weight vectors into the matmul weights, cast to bf16, and
    reshape biases. Returns the per-core common input dict (everything except
    x)."""
    bf = ml_dtypes.bfloat16
    w_mix = np.asarray(inputs["w_rms_mix"], np.float32)[:, None]
    w_ffn = np.asarray(inputs["w_rms_ffn"], np.float32)[:, None]
    KD = cfg.D // 128
    return {
        "wg": (w_mix * np.asarray(inputs["Wg"], np.float32)).astype(bf),
        "wv": (w_mix * np.asarray(inputs["Wv"], np.float32)).astype(bf),
        "wd": (w_mix * np.asarray(inputs["Wd"], np.float32)).astype(bf),
        "bg": np.ascontiguousarray(
            np.asarray(inputs["bg"], np.float32).reshape(KD, 128)
        ),
        "bv": np.ascontiguousarray(
            np.asarray(inputs["bv"], np.float32).reshape(KD, 128)
        ),
        "bd": np.ascontiguousarray(
            np.asarray(inputs["bd"], np.float32).reshape(KD, 128)
        ),
        "wgate": (w_ffn * np.asarray(inputs["W_gate"], np.float32)).astype(bf),
        "wup": (w_ffn * np.asarray(inputs["W_up"], np.float32)).astype(bf),
        "wout": np.asarray(inputs["W_out"], np.float32).astype(bf),
    }


def build_nc(cfg: Cfg):
    bf = mybir.dt.bfloat16
    # Bacc (not bare Bass): its compile() pass splits multi-wait sync into
    # event semaphores (HW allows at most 1 wait per instruction) and
    # hoists ACT table loads.
    nc = bacc.Bacc("TRN2", target_bir_lowering=False, debug=False)
    KD = cfg.D // 128
    ins = {
        "x": nc.declare_dram_parameter("x", [cfg.S, cfg.D], F32, isOutput=False),
        "wg": nc.declare_dram_parameter("wg", [cfg.D, cfg.D], bf, isOutput=False),
        "wv": nc.declare_dram_parameter("wv", [cfg.D, cfg.D], bf, isOutput=False),
        "wd": nc.declare_dram_parameter("wd", [cfg.D, cfg.D], bf, isOutput=False),
        "bg": nc.declare_dram_parameter("bg", [KD, 128], F32, isOutput=False),
        "bv": nc.declare_dram_parameter("bv", [KD, 128], F32, isOutput=False),
        "bd": nc.declare_dram_parameter("bd", [KD, 128], F32, isOutput=False),
        "wgate": nc.declare_dram_parameter("wgate", [cfg.D, cfg.F], bf, isOutput=False),
        "wup": nc.declare_dram_parameter("wup", [cfg.D, cfg.F], bf, isOutput=False),
        "wout": nc.declare_dram_parameter("wout", [cfg.F, cfg.D], bf, isOutput=False),
    }
    outs = {
        "out": nc.declare_dram_parameter("out", [cfg.S, cfg.D], F32, isOutput=True),
    }
    ins_ap = {k: v.ap() for k, v in ins.items()}
    outs_ap = {k: v.ap() for k, v in outs.items()}
    with tile.TileContext(nc, pool_alloc_mode="queue") as tc:
        build_mingru(tc, outs_ap, ins_ap, cfg)
    nc.compile()
    return nc


_NC_CACHE = {}


def kernel(**inputs) -> np.ndarray:
    from concourse.bass_utils import run_bass_kernel_spmd

    cfg = Cfg()
    x = np.asarray(inputs["x"], np.float32)  # [B, S, D]
    B = x.shape[0]
    common = prep_weights(inputs, cfg)

    if cfg not in _NC_CACHE:
        _NC_CACHE[cfg] = build_nc(cfg)
    nc = _NC_CACHE[cfg]

    in_maps = [dict(common, x=np.ascontiguousarray(x[b])) for b in range(B)]
    res = run_bass_kernel_spmd(nc, in_maps, core_ids=list(range(B)))
    out = np.stack([np.asarray(res.results[b]["out"]) for b in range(B)], axis=0)
    return out.astype(np.float32)


def _ensure_ntff_hook():
    """Register the axon NTFF profile hook if the agent image's antenv lacks
    axon_hooks (same ctypes shim trn_boot would install)."""
    import contextlib
    import ctypes
    import types

    try:
        from antenv.axon_hooks import get_axon_ntff_profile_hook

        if get_axon_ntff_profile_hook() is not None:
            return
    except ImportError:
        pass

    so_path = "/opt/axon/libaxon_pjrt.so"
    lib = ctypes.CDLL(so_path)
    if not hasattr(lib, "axon_start_nrt_profile"):
        return
    lib.axon_start_nrt_profile.argtypes = [
        ctypes.POINTER(ctypes.c_int64),
        ctypes.c_size_t,
    ]
    lib.axon_start_nrt_profile.restype = ctypes.c_int64
    lib.axon_stop_nrt_profile.argtypes = [ctypes.c_char_p]
    lib.axon_stop_nrt_profile.restype = ctypes.c_int64

    @contextlib.contextmanager
    def _hook(output_dir, device_ids):
        import jax

        jax.devices()
        if device_ids:
            ids = (ctypes.c_int64 * len(device_ids))(*device_ids)
            rc = lib.axon_start_nrt_profile(ids, len(device_ids))
        else:
            rc = lib.axon_start_nrt_profile(None, 0)
        if rc != 0:
            raise RuntimeError(f"axon_start_nrt_profile rc={rc}")
        try:
            yield
        finally:
            n = lib.axon_stop_nrt_profile(str(output_dir).encode())
            print(f"profile: {n} file(s) written to {output_dir}")

    mod = types.ModuleType("antenv.axon_hooks")
    mod.get_axon_ntff_profile_hook = lambda: _hook
    mod.set_axon_ntff_profile_hook = lambda h: None
    sys.modules["antenv.axon_hooks"] = mod
    import antenv

    antenv.axon_hooks = mod


def kernel_profiled(**inputs):
    """Run once with NTFF tracing; returns exec_time_ns (max across cores)."""
    from concourse import bass_utils
    from concourse.bass_utils import run_bass_kernel_spmd

    _ensure_ntff_hook()
    # skip the bucket upload (no creds needed for local analysis)
    bass_utils.upload_artifacts = lambda tmpdir: f"local:{tmpdir}"

    cfg = Cfg()
    x = np.asarray(inputs["x"], np.float32)
    B = x.shape[0]
    common = prep_weights(inputs, cfg)
    if cfg not in _NC_CACHE:
        _NC_CACHE[cfg] = build_nc(cfg)
    nc = _NC_CACHE[cfg]
    in_maps = [dict(common, x=np.ascontiguousarray(x[b])) for b in range(B)]
    import os
    tmpdir = "/tmp/mingru_profile"
    os.makedirs(tmpdir, exist_ok=True)
    res = run_bass_kernel_spmd(
        nc, in_maps, core_ids=list(range(B)), trace=True, tmpdir=tmpdir
    )
    return res.exec_time_ns


if __name__ == "__main__":
    rng = np.random.default_rng(0)
    cfg = Cfg()
    fake = {
        "x": rng.standard_normal((8, cfg.S, cfg.D), dtype=np.float32),
        "w_rms_mix": np.ones(cfg.D, np.float32),
        "w_rms_ffn": np.ones(cfg.D, np.float32),
        "Wg": rng.standard_normal((cfg.D, cfg.D), dtype=np.float32) / 32,
        "bg": np.zeros(cfg.D, np.float32),
        "Wv": rng.standard_normal((cfg.D, cfg.D), dtype=np.float32) / 32,
        "bv": np.zeros(cfg.D, np.float32),
        "Wd": rng.standard_normal((cfg.D, cfg.D), dtype=np.float32) / 32,
        "bd": np.ones(cfg.D, np.float32),
        "W_gate": rng.standard_normal((cfg.D, cfg.F), dtype=np.float32) / 32,
        "W_up": rng.standard_normal((cfg.D, cfg.F), dtype=np.float32) / 32,
        "W_out": rng.standard_normal((cfg.F, cfg.D), dtype=np.float32) / 55,
    }
    out = kernel(**fake)
    print(out.shape, out.dtype)


# revision 29
# speedup vs baseline: 1.0959x; 1.0097x over previous
"""MinGRU block kernel for Trainium2 (Bass/Tile), 8-core data-parallel over batch.

Reference computation (per batch b):
    xn = rmsnorm(x, w_rms_mix)
    g = xn@Wg+bg; v = xn@Wv+bv; d = xn@Wd+bd
    x_scan = sigmoid(g)*tanh(v);  a = 0.001 + 0.998*sigmoid(d)
    h = linear_scan(x_scan, a)          # h_t = a_t h_{t-1} + x_t along S
    x2 = x + h
    yn = rmsnorm(x2, w_rms_ffn)
    out = x2 + (silu(yn@W_gate) * (yn@W_up)) @ W_out

Shapes: B=8, S=4096, D=1024, F=3072 (fp32).  Each core handles one batch.

Design notes:
  - All matmul activations live in transposed layout [feature, token] so the
    contraction dim (features) is on partitions; weights are used directly as
    lhsT in their natural [in, out] storage.  The sequential scan runs along
    the free (token) axis via the DVE tensor_tensor_scan instruction.
  - Phase 1 (mixer): transpose x (PE), g/v/d matmuls, sigmoid/tanh epilogue,
    scan with carried state across token blocks, x2^T = x^T + h^T, plus the
    FFN-norm sum-of-squares (ones-matmul over partitions).  x2^T spills to
    DRAM.
  - Phase 2a: gate/up matmuls from x2n^T, silu*up -> hidden^T (spills).
  - Phase 2b: out matmul, residual add, PE transpose-back to natural layout.
  - rsqrt for rmsnorm: ACT Sqrt + DVE reciprocal + one Newton step (ACT Rsqrt
    is banned for accuracy).  Sqrt calls are hoisted/batched so the ACT table
    set switches only ~4 times total.
  - Weights are pre-folded with the rms weight vectors and cast to bf16 on
    host; matmuls run in bf16 (fp32 accumulation in PSUM).
"""

import sys

for _p in ("/opt/trn_rl_repo", "/root/.axon_site/_ro/trn_rl_repo"):
    if _p not in sys.path:
        sys.path.insert(0, _p)

from contextlib import ExitStack
from dataclasses import dataclass

import ml_dtypes
import numpy as np

import concourse.bass as bass
import concourse.tile as tile
from concourse import bacc, mybir
from concourse.masks import make_identity

F32 = mybir.dt.float32
BF16 = mybir.dt.bfloat16
AF = mybir.ActivationFunctionType
ALU = mybir.AluOpType

EPS = 1e-6


@dataclass(frozen=True)
class Cfg:
    S: int = 4096
    D: int = 1024
    F: int = 3072
    Tb: int = 256  # token block (matmul moving free dim)

    @property
    def NB(self):
        return self.S // self.Tb

    @property
    def TC(self):
        return self.Tb // 128  # token chunks per block

    @property
    def KD(self):
        return self.D // 128  # D in 128-chunks

    @property
    def KF(self):
        return self.F // 128  # F in 128-chunks


I32 = mybir.dt.int32


def _rsqrt_dve(nc, pool, ms, shape, tag, iters=3):
    """r = 1/sqrt(ms) entirely on the DVE: quake-III magic seed + Newton.

    Avoids ACT Sqrt so no activation-table switches are needed mid-kernel.
    ms is an f32 AP; returns an f32 tile of `shape`.
    """
    ti = pool.tile(shape, I32, tag=f"{tag}_i", name=f"{tag}_i")
    # 0x5f3759df - (i >> 1) == ((i >> 1) ^ -1) + 0x5f3759e0 in two's
    # complement; bitwise and arith ops can't share one tensor_scalar
    nc.vector.tensor_scalar(ti, ms.bitcast(I32), 1, -1,
                            op0=ALU.logical_shift_right, op1=ALU.bitwise_xor)
    nc.vector.tensor_scalar(ti, ti, 0x5F3759E0, None, op0=ALU.add)
    r = pool.tile(shape, F32, tag=f"{tag}_r", name=f"{tag}_r")
    nc.vector.tensor_copy(r, ti.bitcast(F32))
    t1 = pool.tile(shape, F32, tag=f"{tag}_t", name=f"{tag}_t")
    for _ in range(iters):
        nc.vector.tensor_mul(t1, r, r)
        nc.vector.tensor_mul(t1, t1, ms)
        nc.vector.tensor_scalar(t1, t1, -0.5, 1.5, op0=ALU.mult, op1=ALU.add)
        nc.vector.tensor_mul(r, r, t1)
    return r


def build_mingru(tc: tile.TileContext, outs: dict, ins: dict, cfg: Cfg):
    nc = tc.nc
    S, D, F_, Tb = cfg.S, cfg.D, cfg.F, cfg.Tb
    NB, TC, KD, KF = cfg.NB, cfg.TC, cfg.KD, cfg.KF

    x = ins["x"]  # [S, D] f32
    wg, wv, wd = ins["wg"], ins["wv"], ins["wd"]  # [D, D] bf16 (rms-folded)
    bg, bv, bd = ins["bg"], ins["bv"], ins["bd"]  # [KD, 128] f32
    wgate, wup = ins["wgate"], ins["wup"]  # [D, F] bf16 (rms-folded)
    wout = ins["wout"]  # [F, D] bf16
    out = outs["out"]  # [S, D] f32

    ctx = ExitStack()
    with ctx:
        singles = ctx.enter_context(tc.tile_pool(name="singles", bufs=1))
        dram = ctx.enter_context(tc.tile_pool(name="dram", bufs=1, space="DRAM"))

        ident = singles.tile([128, 128], F32)
        make_identity(nc, ident)
        ones_row = singles.tile([1, 128], F32)
        nc.gpsimd.memset(ones_row, 1.0)
        ones_col = singles.tile([128, 1], BF16)
        nc.gpsimd.memset(ones_col, 1.0)
        # pre-warm the sigmoid activation table set while the first DMAs run
        actwarm = singles.tile([1, 1], F32)
        nc.scalar.activation(actwarm, ones_row[0:1, 0:1], AF.Sigmoid)

        # biases as [128, KD] so bias[:, m] is a per-partition scalar AP
        bgs = singles.tile([128, KD], F32)
        bvs = singles.tile([128, KD], F32)
        bds = singles.tile([128, KD], F32)
        nc.sync.dma_start(out=bgs, in_=bg.rearrange("m p -> p m"))
        nc.sync.dma_start(out=bvs, in_=bv.rearrange("m p -> p m"))
        nc.sync.dma_start(out=bds, in_=bd.rearrange("m p -> p m"))

        # DRAM scratch
        x2t_d = dram.tile([D, S], F32)
        rms2_d = dram.tile([1, S], F32)

        # ---------------- phase 1: mixer ----------------
        prev_h = {}
        with tc.tile_pool(name="wmix", bufs=1) as wmix, tc.tile_pool(
            name="p1", bufs=2
        ) as p1, tc.tile_pool(name="p1h", bufs=2) as p1h, tc.tile_pool(
            name="ps_tr", bufs=2, space="PSUM"
        ) as ps_tr, tc.tile_pool(
            name="ps_gvd", bufs=1, space="PSUM"
        ) as ps_gvd, tc.tile_pool(
            name="ps_ss2", bufs=2, space="PSUM"
        ) as ps_ss2:
            # first x block loads before the weights so the transpose/norm
            # pipeline fills while weights stream in
            xblk0 = p1.tile([128, TC, D], F32, tag="xblk", name="xblk0")
            nc.sync.dma_start(
                out=xblk0, in_=x[0:Tb, :].rearrange("(c p) d -> p c d", p=128)
            )

            # mixer weights resident: [128, D] bf16 per k-chunk
            wg_sb = [wmix.tile([128, D], BF16, tag=f"wg{k}", name=f"wg{k}") for k in range(KD)]
            wv_sb = [wmix.tile([128, D], BF16, tag=f"wv{k}", name=f"wv{k}") for k in range(KD)]
            wd_sb = [wmix.tile([128, D], BF16, tag=f"wd{k}", name=f"wd{k}") for k in range(KD)]
            for k in range(KD):
                nc.sync.dma_start(out=wg_sb[k], in_=wg[k * 128 : (k + 1) * 128, :])
                nc.sync.dma_start(out=wv_sb[k], in_=wv[k * 128 : (k + 1) * 128, :])
                nc.sync.dma_start(out=wd_sb[k], in_=wd[k * 128 : (k + 1) * 128, :])

            for j in range(NB):
                t0 = j * Tb
                # load x block as [p, c, d]
                if j == 0:
                    xblk = xblk0
                else:
                    xblk = p1.tile([128, TC, D], F32, tag="xblk", name="xblk")
                    nc.sync.dma_start(
                        out=xblk,
                        in_=x[t0 : t0 + Tb, :].rearrange("(c p) d -> p c d", p=128),
                    )

                # rms1 for this block: ACT square+accum -> DVE rsqrt
                ss1 = p1.tile([128, TC], F32, tag="ss1", name="ss1")
                for c in range(TC):
                    sqdump = p1.tile([128, D], BF16, tag="sqdump", name="sqdump")
                    nc.scalar.activation(
                        sqdump, xblk[:, c, :], AF.Square,
                        accum_out=ss1[:, c : c + 1],
                    )
                nc.vector.tensor_scalar(
                    ss1, ss1, 1.0 / D, EPS, op0=ALU.mult, op1=ALU.add
                )
                rms1 = _rsqrt_dve(nc, p1, ss1, [128, TC], "rms1")
                # transpose to a [1, Tb] row on partition 0, then broadcast
                # down 128 partitions via a K=1 ones matmul
                rowps = ps_ss2.tile([1, Tb], F32, tag="ss2", name="rowps")
                for c in range(TC):
                    nc.tensor.transpose(
                        rowps[0:1, c * 128 : (c + 1) * 128],
                        rms1[:, c : c + 1], ident,
                    )
                rms1row = p1.tile([1, Tb], F32, tag="rms1row", name="rms1row")
                nc.vector.tensor_copy(rms1row, rowps)
                rb = ps_tr.tile([128, Tb], F32, tag="rb", bufs=1, name="rb")
                for c in range(TC):
                    nc.tensor.matmul(
                        rb[:, c * 128 : (c + 1) * 128],
                        lhsT=ones_row,
                        rhs=rms1row[0:1, c * 128 : (c + 1) * 128],
                        start=True,
                        stop=True,
                    )

                # transpose x -> x^T tiles, and xn^T = x^T * rms1 (bf16)
                xT = []
                xnT = []
                for m in range(KD):
                    pt = ps_tr.tile([128, Tb], F32, tag="ptr", name="ptr")
                    for c in range(TC):
                        nc.tensor.transpose(
                            pt[:, c * 128 : (c + 1) * 128],
                            xblk[:, c, m * 128 : (m + 1) * 128],
                            ident,
                        )
                    xT_m = p1.tile([128, Tb], F32, tag=f"xT{m}", name=f"xT{m}")
                    nc.vector.tensor_copy(xT_m, pt)
                    xnT_m = p1.tile([128, Tb], BF16, tag=f"xnT{m}", name=f"xnT{m}")
                    nc.vector.tensor_mul(xnT_m, xT_m, rb)
                    xT.append(xT_m)
                    xnT.append(xnT_m)

                # mixer matmuls + epilogue + scan, per output d-chunk
                for m in range(KD):
                    psg = ps_gvd.tile([128, Tb], F32, tag="psg", name="psg")
                    psv = ps_gvd.tile([128, Tb], F32, tag="psv", name="psv")
                    psd = ps_gvd.tile([128, Tb], F32, tag="psd", name="psd")
                    for k in range(KD):
                        st, sp = (k == 0), (k == KD - 1)
                        nc.tensor.matmul(
                            psg, lhsT=wg_sb[k][:, m * 128 : (m + 1) * 128],
                            rhs=xnT[k], start=st, stop=sp,
                        )
                        nc.tensor.matmul(
                            psv, lhsT=wv_sb[k][:, m * 128 : (m + 1) * 128],
                            rhs=xnT[k], start=st, stop=sp,
                        )
                        nc.tensor.matmul(
                            psd, lhsT=wd_sb[k][:, m * 128 : (m + 1) * 128],
                            rhs=xnT[k], start=st, stop=sp,
                        )
                    sg = p1.tile([128, Tb], F32, tag="sg", name="sg")
                    nc.scalar.activation(sg, psg, AF.Sigmoid, bias=bgs[:, m : m + 1])
                    tv = p1.tile([128, Tb], F32, tag="tv", name="tv")
                    nc.scalar.activation(tv, psv, AF.Tanh, bias=bvs[:, m : m + 1])
                    sd = p1.tile([128, Tb], F32, tag="sd", name="sd")
                    nc.scalar.activation(sd, psd, AF.Sigmoid, bias=bds[:, m : m + 1])

                    xs = p1.tile([128, Tb], F32, tag="xs", name="xs")
                    nc.vector.tensor_mul(xs, sg, tv)
                    aa = p1.tile([128, Tb], F32, tag="aa", name="aa")
                    nc.vector.tensor_scalar(
                        aa, sd, 0.998, 0.001, op0=ALU.mult, op1=ALU.add
                    )

                    h_m = p1h.tile([128, Tb], F32, tag=f"h{m}", name=f"h{m}")
                    init = 0.0 if j == 0 else prev_h[m][:, Tb - 1 : Tb]
                    nc.vector.tensor_tensor_scan(
                        h_m, data0=aa, data1=xs, initial=init,
                        op0=ALU.mult, op1=ALU.add,
                    )
                    prev_h[m] = h_m

                    x2T_m = p1.tile([128, Tb], F32, tag=f"x2T{m}", name=f"x2T{m}")
                    nc.vector.tensor_add(x2T_m, xT[m], h_m)
                    nc.sync.dma_start(
                        out=x2t_d[m * 128 : (m + 1) * 128, t0 : t0 + Tb],
                        in_=x2T_m,
                    )
                    # FFN-norm sum of squares: ones^T @ x2^2 accumulated over m
                    sq2 = p1.tile([128, Tb], BF16, tag="sq2", name="sq2")
                    nc.scalar.activation(sq2, x2T_m, AF.Square)
                    ss2_ps = (
                        ps_ss2.tile([1, Tb], F32, tag="ss2", name="ss2")
                        if m == 0
                        else ss2_ps
                    )
                    nc.tensor.matmul(
                        ss2_ps, lhsT=ones_col, rhs=sq2,
                        start=(m == 0), stop=(m == KD - 1),
                    )
                # rms2 for this block -> DRAM row (used by phase 2a)
                ss2_sb = p1.tile([1, Tb], F32, tag="ss2sb", name="ss2sb")
                nc.vector.tensor_copy(ss2_sb, ss2_ps)
                nc.vector.tensor_scalar(
                    ss2_sb, ss2_sb, 1.0 / D, EPS, op0=ALU.mult, op1=ALU.add
                )
                rms2 = _rsqrt_dve(nc, p1, ss2_sb, [1, Tb], "rms2")
                nc.sync.dma_start(out=rms2_d[:, t0 : t0 + Tb], in_=rms2)

        # ---------------- phase 2: full FFN (gate/up -> hidden -> out) --------
        # hidden stays in SBUF per block (no DRAM spill); x2^T is read once
        # per block and reused for both the norm input and the residual.
        with tc.tile_pool(name="wffn", bufs=1) as wffn, tc.tile_pool(
            name="p2", bufs=2
        ) as p2, tc.tile_pool(name="ps_2", bufs=2, space="PSUM") as ps_2:
            wgate_sb = [wffn.tile([128, F_], BF16, tag=f"wgate{k}", name=f"wgate{k}") for k in range(KD)]
            wup_sb = [wffn.tile([128, F_], BF16, tag=f"wup{k}", name=f"wup{k}") for k in range(KD)]
            wout_sb = [wffn.tile([128, D], BF16, tag=f"wout{k}", name=f"wout{k}") for k in range(KF)]
            # gate/up weights first (they gate the first matmuls), split in
            # F-halves so low-f matmuls can start before the full tiles land;
            # wout's load hides behind the first block's gate/up compute
            H = F_ // 2
            for half in range(2):
                fs = slice(half * H, (half + 1) * H)
                for k in range(KD):
                    nc.sync.dma_start(
                        out=wgate_sb[k][:, fs],
                        in_=wgate[k * 128 : (k + 1) * 128, fs],
                    )
                    nc.sync.dma_start(
                        out=wup_sb[k][:, fs],
                        in_=wup[k * 128 : (k + 1) * 128, fs],
                    )
            for k in range(KF):
                nc.sync.dma_start(out=wout_sb[k], in_=wout[k * 128 : (k + 1) * 128, :])

            for j in range(NB):
                t0 = j * Tb
                rms2row = p2.tile([1, Tb], F32, tag="rms2row", name="rms2row")
                nc.sync.dma_start(out=rms2row, in_=rms2_d[:, t0 : t0 + Tb])
                rb2 = ps_2.tile([128, Tb], F32, tag="rb2", bufs=1, name="rb2")
                for c in range(TC):
                    nc.tensor.matmul(
                        rb2[:, c * 128 : (c + 1) * 128],
                        lhsT=ones_row,
                        rhs=rms2row[0:1, c * 128 : (c + 1) * 128],
                        start=True,
                        stop=True,
                    )
                x2a = []
                x2nT = []
                for m in range(KD):
                    x2a_m = p2.tile([128, Tb], F32, tag=f"x2a{m}", name=f"x2a{m}")
                    nc.sync.dma_start(
                        out=x2a_m,
                        in_=x2t_d[m * 128 : (m + 1) * 128, t0 : t0 + Tb],
                    )
                    x2nT_m = p2.tile([128, Tb], BF16, tag=f"x2nT{m}", name=f"x2nT{m}")
                    nc.vector.tensor_mul(x2nT_m, x2a_m, rb2)
                    x2a.append(x2a_m)
                    x2nT.append(x2nT_m)

                hidden = []
                for f in range(KF):
                    pg = ps_2.tile([128, Tb], F32, tag="pg", name="pg")
                    pu = ps_2.tile([128, Tb], F32, tag="pu", name="pu")
                    for k in range(KD):
                        st, sp = (k == 0), (k == KD - 1)
                        nc.tensor.matmul(
                            pg, lhsT=wgate_sb[k][:, f * 128 : (f + 1) * 128],
                            rhs=x2nT[k], start=st, stop=sp,
                        )
                        nc.tensor.matmul(
                            pu, lhsT=wup_sb[k][:, f * 128 : (f + 1) * 128],
                            rhs=x2nT[k], start=st, stop=sp,
                        )
                    # silu(g) = g * sigmoid(g), composed so each DVE op reads
                    # at most one PSUM operand
                    sl = p2.tile([128, Tb], F32, tag="sl", name="sl")
                    nc.scalar.activation(sl, pg, AF.Sigmoid)
                    sl2 = p2.tile([128, Tb], F32, tag="sl2", name="sl2")
                    nc.vector.tensor_mul(sl2, sl, pg)
                    hid = p2.tile([128, Tb], BF16, tag=f"hid{f}", bufs=1, name=f"hid{f}")
                    nc.vector.tensor_mul(hid, sl2, pu)
                    hidden.append(hid)

                outT = []
                for m in range(KD):
                    pf = ps_2.tile([128, Tb], F32, tag="pf", name="pf")
                    for k in range(KF):
                        nc.tensor.matmul(
                            pf, lhsT=wout_sb[k][:, m * 128 : (m + 1) * 128],
                            rhs=hidden[k], start=(k == 0), stop=(k == KF - 1),
                        )
                    outT_m = p2.tile([128, Tb], F32, tag=f"outT{m}", bufs=1, name=f"outT{m}")
                    nc.vector.tensor_add(outT_m, x2a[m], pf)
                    outT.append(outT_m)
                # transpose back to natural [token, D] and store
                for c in range(TC):
                    obl = p2.tile([128, D], F32, tag="obl", bufs=3, name="obl")
                    for m in range(KD):
                        pt2 = ps_2.tile([128, 128], F32, tag="pt2", bufs=1, name="pt2")
                        nc.tensor.transpose(
                            pt2, outT[m][:, c * 128 : (c + 1) * 128], ident
                        )
                        nc.vector.tensor_copy(obl[:, m * 128 : (m + 1) * 128], pt2)
                    tt = t0 + c * 128
                    nc.sync.dma_start(out=out[tt : tt + 128, :], in_=obl)


# ----------------------------------------------------------------------------
# host side
# ----------------------------------------------------------------------------

def prep_weights(inputs: dict, cfg: Cfg):
    """Fold rms weight vectors into the matmul weights, cast to bf16, and
    reshape biases. Returns the per-core common input dict (everything except
    x)."""
    bf = ml_dtypes.bfloat16
    w_mix = np.asarray(inputs["w_rms_mix"], np.float32)[:, None]
    w_ffn = np.asarray(inputs["w_rms_ffn"], np.float32)[:, None]
    KD = cfg.D // 128
    return {
        "wg": (w_mix * np.asarray(inputs["Wg"], np.float32)).astype(bf),
        "wv": (w_mix * np.asarray(inputs["Wv"], np.float32)).astype(bf),
        "wd": (w_mix * np.asarray(inputs["Wd"], np.float32)).astype(bf),
        "bg": np.ascontiguousarray(
            np.asarray(inputs["bg"], np.float32).reshape(KD, 128)
        ),
        "bv": np.ascontiguousarray(
            np.asarray(inputs["bv"], np.float32).reshape(KD, 128)
        ),
        "bd": np.ascontiguousarray(
            np.asarray(inputs["bd"], np.float32).reshape(KD, 128)
        ),
        "wgate": (w_ffn * np.asarray(inputs["W_gate"], np.float32)).astype(bf),
        "wup": (w_ffn * np.asarray(inputs["W_up"], np.float32)).astype(bf),
        "wout": np.asarray(inputs["W_out"], np.float32).astype(bf),
    }


def build_nc(cfg: Cfg):
    bf = mybir.dt.bfloat16
    # Bacc (not bare Bass): its compile() pass splits multi-wait sync into
    # event semaphores (HW allows at most 1 wait per instruction) and
    # hoists ACT table loads.
    nc = bacc.Bacc("TRN2", target_bir_lowering=False, debug=False)
    KD = cfg.D // 128
    ins = {
        "x": nc.declare_dram_parameter("x", [cfg.S, cfg.D], F32, isOutput=False),
        "wg": nc.declare_dram_parameter("wg", [cfg.D, cfg.D], bf, isOutput=False),
        "wv": nc.declare_dram_parameter("wv", [cfg.D, cfg.D], bf, isOutput=False),
        "wd": nc.declare_dram_parameter("wd", [cfg.D, cfg.D], bf, isOutput=False),
        "bg": nc.declare_dram_parameter("bg", [KD, 128], F32, isOutput=False),
        "bv": nc.declare_dram_parameter("bv", [KD, 128], F32, isOutput=False),
        "bd": nc.declare_dram_parameter("bd", [KD, 128], F32, isOutput=False),
        "wgate": nc.declare_dram_parameter("wgate", [cfg.D, cfg.F], bf, isOutput=False),
        "wup": nc.declare_dram_parameter("wup", [cfg.D, cfg.F], bf, isOutput=False),
        "wout": nc.declare_dram_parameter("wout", [cfg.F, cfg.D], bf, isOutput=False),
    }
    outs = {
        "out": nc.declare_dram_parameter("out", [cfg.S, cfg.D], F32, isOutput=True),
    }
    ins_ap = {k: v.ap() for k, v in ins.items()}
    outs_ap = {k: v.ap() for k, v in outs.items()}
    with tile.TileContext(nc, pool_alloc_mode="queue") as tc:
        build_mingru(tc, outs_ap, ins_ap, cfg)
    nc.compile()
    return nc


_NC_CACHE = {}


def kernel(**inputs) -> np.ndarray:
    from concourse.bass_utils import run_bass_kernel_spmd

    cfg = Cfg()
    x = np.asarray(inputs["x"], np.float32)  # [B, S, D]
    B = x.shape[0]
    common = prep_weights(inputs, cfg)

    if cfg not in _NC_CACHE:
        _NC_CACHE[cfg] = build_nc(cfg)
    nc = _NC_CACHE[cfg]

    in_maps = [dict(common, x=np.ascontiguousarray(x[b])) for b in range(B)]
    res = run_bass_kernel_spmd(nc, in_maps, core_ids=list(range(B)))
    out = np.stack([np.asarray(res.results[b]["out"]) for b in range(B)], axis=0)
    return out.astype(np.float32)


def _ensure_ntff_hook():
    """Register the axon NTFF profile hook if the agent image's antenv lacks
    axon_hooks (same ctypes shim trn_boot would install)."""
    import contextlib
    import ctypes
    import types

    try:
        from antenv.axon_hooks import get_axon_ntff_profile_hook

        if get_axon_ntff_profile_hook() is not None:
            return
    except ImportError:
        pass

    so_path = "/opt/axon/libaxon_pjrt.so"
    lib = ctypes.CDLL(so_path)
    if not hasattr(lib, "axon_start_nrt_profile"):
        return
    lib.axon_start_nrt_profile.argtypes = [
        ctypes.POINTER(ctypes.c_int64),
        ctypes.c_size_t,
    ]
    lib.axon_start_nrt_profile.restype = ctypes.c_int64
    lib.axon_stop_nrt_profile.argtypes = [ctypes.c_char_p]
    lib.axon_stop_nrt_profile.restype = ctypes.c_int64

    @contextlib.contextmanager
    def _hook(output_dir, device_ids):
        import jax

        jax.devices()
        if device_ids:
            ids = (ctypes.c_int64 * len(device_ids))(*device_ids)
            rc = lib.axon_start_nrt_profile(ids, len(device_ids))
        else:
            rc = lib.axon_start_nrt_profile(None, 0)
        if rc != 0:
            raise RuntimeError(f"axon_start_nrt_profile rc={rc}")
        try:
            yield
        finally:
            n = lib.axon_stop_nrt_profile(str(output_dir).encode())
            print(f"profile: {n} file(s) written to {output_dir}")

    mod = types.ModuleType("antenv.axon_hooks")
    mod.get_axon_ntff_profile_hook = lambda: _hook
    mod.set_axon_ntff_profile_hook = lambda h: None
    sys.modules["antenv.axon_hooks"] = mod
    import antenv

    antenv.axon_hooks = mod


def kernel_profiled(**inputs):
    """Run once with NTFF tracing; returns exec_time_ns (max across cores)."""
    from concourse import bass_utils
    from concourse.bass_utils import run_bass_kernel_spmd

    _ensure_ntff_hook()
    # skip the bucket upload (no creds needed for local analysis)
    bass_utils.upload_artifacts = lambda tmpdir: f"local:{tmpdir}"

    cfg = Cfg()
    x = np.asarray(inputs["x"], np.float32)
    B = x.shape[0]
    common = prep_weights(inputs, cfg)
    if cfg not in _NC_CACHE:
        _NC_CACHE[cfg] = build_nc(cfg)
    nc = _NC_CACHE[cfg]
    in_maps = [dict(common, x=np.ascontiguousarray(x[b])) for b in range(B)]
    import os
    tmpdir = "/tmp/mingru_profile"
    os.makedirs(tmpdir, exist_ok=True)
    res = run_bass_kernel_spmd(
        nc, in_maps, core_ids=list(range(B)), trace=True, tmpdir=tmpdir
    )
    return res.exec_time_ns


if __name__ == "__main__":
    rng = np.random.default_rng(0)
    cfg = Cfg()
    fake = {
        "x": rng.standard_normal((8, cfg.S, cfg.D), dtype=np.float32),
        "w_rms_mix": np.ones(cfg.D, np.float32),
        "w_rms_ffn": np.ones(cfg.D, np.float32),
        "Wg": rng.standard_normal((cfg.D, cfg.D), dtype=np.float32) / 32,
        "bg": np.zeros(cfg.D, np.float32),
        "Wv": rng.standard_normal((cfg.D, cfg.D), dtype=np.float32) / 32,
        "bv": np.zeros(cfg.D, np.float32),
        "Wd": rng.standard_normal((cfg.D, cfg.D), dtype=np.float32) / 32,
        "bd": np.ones(cfg.D, np.float32),
        "W_gate": rng.standard_normal((cfg.D, cfg.F), dtype=np.float32) / 32,
        "W_up": rng.standard_normal((cfg.D, cfg.F), dtype=np.float32) / 32,
        "W_out": rng.standard_normal((cfg.F, cfg.D), dtype=np.float32) / 55,
    }
    out = kernel(**fake)
    print(out.shape, out.dtype)
